# revision 2
# baseline (speedup 1.0000x reference)
import numpy as np
import ml_dtypes
import bass_rust as _br

import concourse.bass as bass
import concourse.mybir as mybir
import concourse.tile as tile
from concourse import bacc
from concourse.bass_utils import run_bass_kernel_spmd

F32 = mybir.dt.float32
BF16 = mybir.dt.bfloat16
AX = mybir.AxisListType
AF = mybir.ActivationFunctionType
OP = mybir.AluOpType

B, T, HWs, D, N, C = 16, 64, 7, 768, 12, 64
HW2 = HWs * HWs                 # 49
NCORES = 8
BLOC = B // NCORES              # 2
ROWS = BLOC * T * HW2           # 6272
NCOL = 3 * D                    # 2304
NCH = NCOL // 128               # 18 qkvT chunks
KT = D // 128                   # 6
EPS = 1e-5
SCALE = C ** -0.5
CGRPS = [(g * 512, 512) for g in range(12)] + [(6144, 128)]
G3 = [CGRPS[0:3], CGRPS[3:6], CGRPS[6:9], CGRPS[9:12], CGRPS[12:13]]
XCHUNKS = [(0, 512), (512, 1536), (2048, 2048), (4096, 2176)]

_cache = {}
LAST_RESULT = None


def _build_nc():
    nc = bacc.Bacc(None, target_bir_lowering=False, debug=False)
    xd = nc.declare_dram_parameter("xd", [ROWS, D], BF16, isOutput=False)
    Wqc = nc.declare_dram_parameter("Wqc", [NCH, 128, D], BF16, isOutput=False)
    Wpb = nc.declare_dram_parameter("Wpb", [3, 128, HW2 * 128], BF16, isOutput=False)
    Rpt = nc.declare_dram_parameter("Rpt", [128, 127], F32, isOutput=False)
    affG = nc.declare_dram_parameter("affG", [128, 384], F32, isOutput=False)
    affB = nc.declare_dram_parameter("affB", [128, 384], F32, isOutput=False)
    pbias = nc.declare_dram_parameter("pbias", [128, 3], F32, isOutput=False)
    Wpr = nc.declare_dram_parameter("Wpr", [KT, 128, D], F32, isOutput=False)
    bprojR = nc.declare_dram_parameter("bprojR", [128, D], F32, isOutput=False)
    idtf = nc.declare_dram_parameter("idtf", [128, 128], F32, isOutput=False)
    idtb = nc.declare_dram_parameter("idtb", [128, 128], BF16, isOutput=False)
    yout = nc.declare_dram_parameter("y", [128, D], F32, isOutput=True)

    wpcache = {}
    with tile.TileContext(nc) as tc:
        with (
            tc.tile_pool(name="cst", bufs=1) as cst,
            tc.tile_pool(name="qkv", bufs=1) as qkv,
            tc.tile_pool(name="drp", bufs=8, space="DRAM") as drp,
        ):
            c_idb = cst.tile([128, 128], BF16, tag="idb")
            nc.gpsimd.dma_start(c_idb[:], idtb[:, :])
            c_rpt = cst.tile([128, 127], F32, tag="rpt")
            nc.gpsimd.dma_start(c_rpt[:], Rpt[:, :])
            c_ag = cst.tile([128, 384], F32, tag="ag")
            nc.gpsimd.dma_start(c_ag[:], affG[:, :])
            c_ab = cst.tile([128, 384], F32, tag="ab")
            nc.gpsimd.dma_start(c_ab[:], affB[:, :])
            c_pb = cst.tile([128, 3], F32, tag="pb")
            nc.gpsimd.dma_start(c_pb[:], pbias[:, :])
            c_id = cst.tile([128, 128], F32, tag="idf")
            nc.gpsimd.dma_start(c_id[:], idtf[:, :])
            c_eps = cst.tile([128, 1], F32, tag="eps")
            nc.vector.memset(c_eps[:], EPS)

            atp = tc.alloc_tile_pool(name="atp", bufs=4)
            qT = [qkv.tile([128, 128], F32, tag=f"q{i}", name=f"q{i}") for i in range(6)]
            kTt = [qkv.tile([128, 128], F32, tag=f"k{i}", name=f"k{i}") for i in range(6)]
            vrow = [qkv.tile([128, 128], F32, tag=f"v{i}", name=f"v{i}") for i in range(6)]
            vre = [qkv.tile([64, 256], F32, tag=f"vr{i}", name=f"vr{i}") for i in range(6)]
            outT = [qkv.tile([128, 128], F32, tag=f"o{i}", name=f"o{i}") for i in range(6)]

            # ---------------- stages A/B/C/D fused ----------------
            with (
                tc.tile_pool(name="xtp", bufs=1) as xtp,
                tc.tile_pool(name="qkp", bufs=1) as qkp,
                tc.tile_pool(name="wqp", bufs=2) as wqp,
                tc.tile_pool(name="wpp", bufs=2) as wpp,
                tc.tile_pool(name="ptp", bufs=2) as ptp,
                tc.tile_pool(name="lnt", bufs=2) as lnt,
                tc.tile_pool(name="psB", bufs=1, space="PSUM") as psB,
                tc.tile_pool(name="psC", bufs=1, space="PSUM") as psC,
                tc.tile_pool(name="psD", bufs=1, space="PSUM") as psD,
            ):
                # A: load x row tiles and transpose on PE, 4 tiles per
                # psum bank, contiguous evacuation
                xT = [xtp.tile([128, ROWS], BF16, tag=f"x{kk}", name=f"x{kk}")
                      for kk in range(KT)]
                xip = tc.alloc_tile_pool(name="xin", bufs=3)
                ei0 = 0
                for mb in range(0, HW2, 4):
                    nm = min(4, HW2 - mb)
                    xrow = xip.tile([128, nm * D], BF16, tag="xrow", name="xrow")
                    src = xd[mb * 128:(mb + nm) * 128, :].rearrange(
                        "(blk r) c -> r blk c", blk=nm)
                    nc.sync.dma_start(
                        xrow[:].rearrange("r (blk c) -> r blk c", blk=nm), src)
                    for kk in range(KT):
                        tg = "ptr" if ei0 % 2 == 0 else "ptr2"
                        ptx = psD.tile([128, nm * 128], BF16, tag=tg, name="ptx")
                        for i in range(nm):
                            nc.tensor.transpose(
                                ptx[:, i * 128:(i + 1) * 128],
                                xrow[:, i * D + kk * 128:i * D + (kk + 1) * 128],
                                c_idb[:])
                        dst = xT[kk][:, mb * 128:(mb + nm) * 128]
                        if ei0 % 2 == 0:
                            nc.vector.tensor_copy(dst, ptx[:])
                        else:
                            nc.scalar.copy(dst, ptx[:])
                        ei0 += 1
                xip.release()

                ei = 0
                sS = []
                gds = []
                first_evac = {}

                def emit_pass2(c2, bb, pspool, tagT, tagR, sbp):
                    bsl = slice(bb * 64, (bb + 1) * 64)
                    idx = c2 * 2 + bb
                    rel = sbp.tile([128, 64], F32, tag="rel", name="rel")
                    src = gds[idx][:, :].copy()
                    src.ap = _br.VecI64Pair([[64 * 127, 2], [126, 64], [1, 64]])
                    src.offset = src.offset + 63
                    nc.sync.dma_start(rel[:, :], src)
                    s2 = sbp.tile([128, 64], F32, tag="s2", name="s2")
                    nc.vector.tensor_add(s2[:], sS[idx][:], rel[:])
                    ex = sbp.tile([128, 64], F32, tag="ex", name="ex")
                    den = sbp.tile([128, 1], F32, tag="den", name="den")
                    nc.scalar.activation(ex[:], s2[:], AF.Exp,
                                         bias=0.0, scale=1.0, accum_out=den[:])
                    rden = sbp.tile([128, 1], F32, tag="rden", name="rden")
                    nc.vector.reciprocal(rden[:], den[:])
                    exn = sbp.tile([128, 64], F32, tag="exn", name="exn")
                    nc.vector.tensor_scalar_mul(exn[:], ex[:], rden[:])
                    pTT = pspool.tile([128, 64], F32, tag=tagT, name="pTT")
                    for h in range(2):
                        hsl = slice(h * 64, (h + 1) * 64)
                        pTr = pspool.tile([64, 64], F32, tag=tagR, name="pTr")
                        nc.tensor.transpose(pTr[:], exn[hsl, :], c_id[hsl, hsl])
                        aTh = sbp.tile([64, 64], F32, tag="aT", name="aTh")
                        nc.vector.tensor_copy(aTh[:], pTr[:])
                        co = (h * 2 + bb) * 64
                        nc.tensor.matmul(pTT[hsl, :], vre[c2][:, co:co + 64],
                                         aTh[:], start=True, stop=True)
                    nc.vector.tensor_add(outT[c2][:, bsl], pTT[:, :],
                                         qT[c2][:, bsl])

                for sg in range(6):
                    j = sg // 2
                    # B: qkv projection for 3 chunks
                    qks = [qkp.tile([128, ROWS], BF16, tag=f"qk{i}", name=f"qk{i}")
                           for i in range(3)]
                    for i in range(3):
                        ch = sg * 3 + i
                        wqa = wqp.tile([128, D], BF16, tag="wqa")
                        nc.gpsimd.dma_start(wqa[:], Wqc[ch])
                        for grp in G3:
                            pts = []
                            for t, (goff, gsz) in enumerate(grp):
                                pts.append(psB.tile([128, gsz], F32,
                                                    tag=f"pb{t}", name=f"pb{t}"))
                            for t, (goff, gsz) in enumerate(grp):
                                for kk in range(KT):
                                    nc.tensor.matmul(
                                        pts[t][:], wqa[:, kk * 128:(kk + 1) * 128],
                                        xT[kk][:, goff:goff + gsz],
                                        start=(kk == 0), stop=(kk == KT - 1))
                            for t, (goff, gsz) in enumerate(grp):
                                if ei % 2 == 0:
                                    ev = nc.vector.tensor_copy(
                                        qks[i][:, goff:goff + gsz], pts[t][:])
                                else:
                                    ev = nc.scalar.copy(
                                        qks[i][:, goff:goff + gsz], pts[t][:])
                                if ch not in first_evac:
                                    first_evac[ch] = ev
                                ei += 1
                    # C: pooling conv, accumulate over hw
                    pas = [psC.tile([128, 128], F32, tag=f"pa{i}", name=f"pa{i}")
                           for i in range(3)]
                    if sg % 2 == 0:
                        wpa = wpp.tile([128, 3200], BF16, tag="wpa")
                        nc.gpsimd.dma_start(wpa[:], Wpb[j][:, 0:3200])
                        wpb2 = wpp.tile([128, 3072], BF16, tag="wpb2")
                        nc.gpsimd.dma_start(wpb2[:], Wpb[j][:, 3200:6272])
                        wpcache[j] = (wpa, wpb2)
                    else:
                        wpa, wpb2 = wpcache[j]
                    for hw in range(HW2):
                        if hw < 25:
                            wp = wpa[:, hw * 128:(hw + 1) * 128]
                        else:
                            wp = wpb2[:, (hw - 25) * 128:(hw - 24) * 128]
                        for i in range(3):
                            nc.tensor.matmul(
                                pas[i][:], wp,
                                qks[i][:, hw:ROWS:HW2],
                                start=(hw == 0), stop=(hw == HW2 - 1))
                    # D: bias + layernorm per chunk, inline
                    for i in range(3):
                        ch = sg * 3 + i
                        i6 = ch % 6
                        pt0 = ptp.tile([128, 128], F32, tag="pt0")
                        nc.vector.tensor_scalar_add(pt0[:], pas[i][:],
                                                    c_pb[:, j:j + 1])
                        p1 = psD.tile([128, 128], F32, tag="ptr")
                        nc.tensor.transpose(p1[:], pt0[:], c_id[:])
                        row = lnt.tile([128, 128], F32, tag="row")
                        if ch % 2 == 0:
                            nc.vector.tensor_copy(row[:], p1[:])
                        else:
                            nc.scalar.copy(row[:], p1[:])
                        rs = lnt.tile([128, 2], F32, tag="rs")
                        for h in range(2):
                            nc.vector.tensor_reduce(
                                rs[:, h:h + 1], row[:, h * 64:(h + 1) * 64],
                                axis=AX.X, op=OP.add)
                        mean = lnt.tile([128, 2], F32, tag="mean")
                        nc.vector.tensor_scalar_mul(mean[:], rs[:], 1.0 / 64)
                        cen = lnt.tile([128, 128], F32, tag="cen")
                        for h in range(2):
                            nc.vector.tensor_scalar_sub(
                                cen[:, h * 64:(h + 1) * 64],
                                row[:, h * 64:(h + 1) * 64], mean[:, h:h + 1])
                        sq = lnt.tile([128, 128], F32, tag="sq")
                        nc.vector.tensor_mul(sq[:], cen[:], cen[:])
                        vs = lnt.tile([128, 2], F32, tag="vs")
                        for h in range(2):
                            nc.vector.tensor_reduce(
                                vs[:, h:h + 1], sq[:, h * 64:(h + 1) * 64],
                                axis=AX.X, op=OP.add)
                        std = lnt.tile([128, 2], F32, tag="std")
                        nc.scalar.activation(std[:], vs[:], AF.Sqrt,
                                             bias=c_eps[:], scale=1.0 / 64)
                        rstd = lnt.tile([128, 2], F32, tag="rstd")
                        nc.vector.reciprocal(rstd[:], std[:])
                        nrm = lnt.tile([128, 128], F32, tag="nrm")
                        for h in range(2):
                            nc.vector.tensor_scalar_mul(
                                nrm[:, h * 64:(h + 1) * 64],
                                cen[:, h * 64:(h + 1) * 64], rstd[:, h:h + 1])
                        tmp = lnt.tile([128, 128], F32, tag="tmp")
                        nc.vector.tensor_mul(tmp[:], nrm[:],
                                             c_ag[:, j * 128:(j + 1) * 128])
                        if j == 2:
                            nc.vector.tensor_add(vrow[i6][:], tmp[:],
                                                 c_ab[:, j * 128:(j + 1) * 128])
                            for h in range(2):
                                for bb in range(2):
                                    co = (h * 2 + bb) * 64
                                    nc.sync.dma_start(
                                        vre[i6][:, co:co + 64],
                                        vrow[i6][bb * 64:(bb + 1) * 64,
                                                 h * 64:(h + 1) * 64])
                        else:
                            fin = lnt.tile([128, 128], F32, tag="fin")
                            nc.vector.tensor_add(fin[:], tmp[:],
                                                 c_ab[:, j * 128:(j + 1) * 128])
                            p2 = psD.tile([128, 128], F32, tag="ptr2")
                            nc.tensor.transpose(p2[:], fin[:], c_id[:])
                            dst = qT[i6] if j == 0 else kTt[i6]
                            if ch % 2 == 0:
                                nc.vector.tensor_copy(dst[:], p2[:])
                            else:
                                nc.scalar.copy(dst[:], p2[:])
                    if sg == 4:
                        for c2e in range(3):
                            for bbe in range(2):
                                emit_pass2(c2e, bbe, psD, "ptr2", "ptr", atp)
                    if sg == 3:
                        # attention pass 1: S = q.k, G = q.rpt (needs only q, k)
                        for c2 in range(6):
                            for bb in range(2):
                                bsl = slice(bb * 64, (bb + 1) * 64)
                                pSG = psD.tile([128, 192], F32, tag="ptr2", name="pSG")
                                for h in range(2):
                                    hsl = slice(h * 64, (h + 1) * 64)
                                    nc.tensor.matmul(pSG[hsl, 0:64],
                                                     qT[c2][hsl, bsl],
                                                     kTt[c2][hsl, bsl],
                                                     start=True, stop=True)
                                    nc.tensor.matmul(pSG[hsl, 64:191],
                                                     qT[c2][hsl, bsl],
                                                     c_rpt[hsl, :],
                                                     start=True, stop=True)
                                sSt = atp.tile([128, 64], F32, tag=f"sS{c2}_{bb}",
                                               name=f"sS{c2}_{bb}", bufs=1)
                                nc.vector.tensor_copy(sSt[:], pSG[:, 0:64])
                                sS.append(sSt)
                                gsb = atp.tile([128, 127], F32, tag="gsb")
                                nc.scalar.copy(gsb[:], pSG[:, 64:191])
                                gd = drp.tile([128, 127], F32, tag="gd")
                                nc.sync.dma_start(gd[:], gsb[:])
                                gds.append(gd)

            # ---------------- stage E: attention pass 2 + projection ----------------
            with (
                tc.tile_pool(name="psO", bufs=3, space="PSUM") as psO,
                tc.tile_pool(name="att", bufs=6) as att,
                tc.tile_pool(name="prj", bufs=1) as prj,
                tc.tile_pool(name="psY", bufs=1, space="PSUM") as psY,
            ):
                c_wpr = []
                for cc in range(KT):
                    w = prj.tile([128, D], F32, tag=f"wpr{cc}", name=f"wpr{cc}")
                    nc.gpsimd.dma_start(w[:], Wpr[cc])
                    c_wpr.append(w)
                c_bpr = prj.tile([128, D], F32, tag="bpr")
                nc.gpsimd.dma_start(c_bpr[:], bprojR[:, :])
                ysb = prj.tile([128, D], F32, tag="ysb")
                for c2 in range(3, 6):
                    for bb in range(2):
                        emit_pass2(c2, bb, psO, "pTT", "pTr", att)

                # projection (accumulates per chunk as outT completes)
                for goff, gsz in [(0, 512), (512, 256)]:
                    pY = psY.tile([128, gsz], F32, tag="pY")
                    for cc in range(KT):
                        nc.tensor.matmul(pY[:], outT[cc][:],
                                         c_wpr[cc][:, goff:goff + gsz],
                                         start=(cc == 0), stop=(cc == KT - 1))
                    nc.vector.tensor_add(ysb[:, goff:goff + gsz], pY[:],
                                         c_bpr[:, goff:goff + gsz])
                    nc.sync.dma_start(yout[:, goff:goff + gsz],
                                      ysb[:, goff:goff + gsz])
            atp.release()

    nc.compile()
    return nc


def _host_prep(W_qkv, Wpq, bpq, Wpk, bpk, Wpv, bpv,
               g_q, be_q, g_k, be_k, g_v, be_v, rel_pos_t, W_proj, b_proj):
    bf = ml_dtypes.bfloat16
    Wqc = np.ascontiguousarray(
        np.asarray(W_qkv, np.float32).reshape(KT, 128, NCH, 128)
        .transpose(2, 1, 0, 3).reshape(NCH, 128, D)).astype(bf)
    Wpb = np.zeros((3, HW2, 128, 128), np.float32)
    for j, Wp in enumerate((Wpq, Wpk, Wpv)):
        Wp = np.asarray(Wp, np.float32)                      # (dout, cin, 7, 7)
        WpT = Wp.transpose(2, 3, 1, 0).reshape(HW2, C, C)    # (hw, ci, dout)
        Wpb[j, :, 0:64, 0:64] = WpT
        Wpb[j, :, 64:128, 64:128] = WpT
    Wpb = np.ascontiguousarray(
        Wpb.transpose(0, 2, 1, 3).reshape(3, 128, HW2 * 128)).astype(bf)
    rp = np.ascontiguousarray(np.asarray(rel_pos_t, np.float32)[::-1].T)  # (64,127)
    Rpt = np.concatenate([rp, rp], axis=0)                   # (128,127)

    gq, gk, gv = (np.asarray(a, np.float32) for a in (g_q, g_k, g_v))
    bq, bk, bv = (np.asarray(a, np.float32) for a in (be_q, be_k, be_v))
    gk = gk * SCALE
    bk = bk * SCALE
    affG = np.broadcast_to(
        np.concatenate([np.tile(g, 2) for g in (gq, gk, gv)])[None, :],
        (128, 384)).copy()
    affB = np.broadcast_to(
        np.concatenate([np.tile(b, 2) for b in (bq, bk, bv)])[None, :],
        (128, 384)).copy()
    pbias = np.ascontiguousarray(np.stack(
        [np.tile(np.asarray(b, np.float32), 2) for b in (bpq, bpk, bpv)], axis=1))
    Wpr = np.ascontiguousarray(
        np.asarray(W_proj, np.float32).reshape(KT, 128, D))
    bprojR = np.broadcast_to(
        np.asarray(b_proj, np.float32)[None, :], (128, D)).copy()
    idtf = np.eye(128, dtype=np.float32)
    return {"Wqc": Wqc, "Wpb": Wpb, "Rpt": Rpt, "affG": affG, "affB": affB,
            "pbias": pbias, "Wpr": Wpr, "bprojR": bprojR, "idtf": idtf,
            "idtb": idtf.astype(bf)}


def kernel(x, W_qkv, Wpq, bpq, Wpk, bpk, Wpv, bpv,
           g_q, be_q, g_k, be_k, g_v, be_v, rel_pos_t, W_proj, b_proj):
    global LAST_RESULT
    if "nc" not in _cache:
        _cache["nc"] = _build_nc()
    nc = _cache["nc"]

    shared = _host_prep(W_qkv, Wpq, bpq, Wpk, bpk, Wpv, bpv,
                        g_q, be_q, g_k, be_k, g_v, be_v,
                        rel_pos_t, W_proj, b_proj)
    bf = ml_dtypes.bfloat16
    xr = np.asarray(x, np.float32).reshape(B, T, HW2, D)
    in_maps = []
    for i in range(NCORES):
        xs = xr[i * BLOC:(i + 1) * BLOC].reshape(ROWS, D).astype(bf)
        m = {"xd": xs}
        m.update(shared)
        in_maps.append(m)

    res = run_bass_kernel_spmd(nc, in_maps, core_ids=list(range(NCORES)))
    LAST_RESULT = res
    y = np.stack([np.asarray(res.results[i]["y"], np.float32)
                  for i in range(NCORES)], axis=0)      # (8, 128, 768)
    return y.reshape(B, T, D).astype(np.float32)


# revision 3
# speedup vs baseline: 1.0537x; 1.0537x over previous
import numpy as np
import ml_dtypes
import bass_rust as _br

import concourse.bass as bass
import concourse.mybir as mybir
import concourse.tile as tile
from concourse import bacc
from concourse.bass_utils import run_bass_kernel_spmd
from concourse.tile_rust import add_dep_helper

F32 = mybir.dt.float32
BF16 = mybir.dt.bfloat16
AX = mybir.AxisListType
AF = mybir.ActivationFunctionType
OP = mybir.AluOpType

B, T, HWs, D, N, C = 16, 64, 7, 768, 12, 64
HW2 = HWs * HWs                 # 49
NCORES = 8
BLOC = B // NCORES              # 2
ROWS = BLOC * T * HW2           # 6272
NCOL = 3 * D                    # 2304
NCH = NCOL // 128               # 18 qkvT chunks
KT = D // 128                   # 6
EPS = 1e-5
SCALE = C ** -0.5
CGRPS = [(g * 512, 512) for g in range(12)] + [(6144, 128)]
G3 = [CGRPS[0:3], CGRPS[3:6], CGRPS[6:9], CGRPS[9:12], CGRPS[12:13]]
XCHUNKS = [(0, 512), (512, 1536), (2048, 2048), (4096, 2176)]

_cache = {}
LAST_RESULT = None


def _build_nc():
    nc = bacc.Bacc(None, target_bir_lowering=False, debug=False)
    xd = nc.declare_dram_parameter("xd", [KT, 128, ROWS], BF16, isOutput=False)
    Wqc = nc.declare_dram_parameter("Wqc", [NCH, 128, D], BF16, isOutput=False)
    Wpb = nc.declare_dram_parameter("Wpb", [3, 128, HW2 * 128], BF16, isOutput=False)
    Rpt = nc.declare_dram_parameter("Rpt", [128, 127], F32, isOutput=False)
    affG = nc.declare_dram_parameter("affG", [128, 384], F32, isOutput=False)
    affB = nc.declare_dram_parameter("affB", [128, 384], F32, isOutput=False)
    pbias = nc.declare_dram_parameter("pbias", [128, 3], F32, isOutput=False)
    Wpr = nc.declare_dram_parameter("Wpr", [KT, 128, D], F32, isOutput=False)
    bprojR = nc.declare_dram_parameter("bprojR", [128, D], F32, isOutput=False)
    idtf = nc.declare_dram_parameter("idtf", [128, 128], F32, isOutput=False)
    yout = nc.declare_dram_parameter("y", [128, D], F32, isOutput=True)

    wpcache = {}
    with tile.TileContext(nc) as tc:
        with (
            tc.tile_pool(name="cst", bufs=1) as cst,
            tc.tile_pool(name="qkv", bufs=1) as qkv,
            tc.tile_pool(name="drp", bufs=8, space="DRAM") as drp,
        ):
            c_rpt = cst.tile([128, 127], F32, tag="rpt")
            nc.scalar.dma_start(c_rpt[:], Rpt[:, :])
            c_ag = cst.tile([128, 384], F32, tag="ag")
            nc.scalar.dma_start(c_ag[:], affG[:, :])
            c_ab = cst.tile([128, 384], F32, tag="ab")
            nc.scalar.dma_start(c_ab[:], affB[:, :])
            c_pb = cst.tile([128, 3], F32, tag="pb")
            nc.scalar.dma_start(c_pb[:], pbias[:, :])
            c_id = cst.tile([128, 128], F32, tag="idf")
            nc.scalar.dma_start(c_id[:], idtf[:, :])
            c_eps = cst.tile([128, 1], F32, tag="eps")
            nc.vector.memset(c_eps[:], EPS)

            atp = tc.alloc_tile_pool(name="atp", bufs=4)
            qT = [qkv.tile([128, 128], F32, tag=f"q{i}", name=f"q{i}") for i in range(6)]
            kTt = [qkv.tile([128, 128], F32, tag=f"k{i}", name=f"k{i}") for i in range(6)]
            vrow = [qkv.tile([128, 128], F32, tag=f"v{i}", name=f"v{i}") for i in range(6)]
            vre = [qkv.tile([64, 256], F32, tag=f"vr{i}", name=f"vr{i}") for i in range(6)]
            outT = [qkv.tile([128, 128], F32, tag=f"o{i}", name=f"o{i}") for i in range(6)]

            # ---------------- stages A/B/C/D fused ----------------
            with (
                tc.tile_pool(name="xtp", bufs=1) as xtp,
                tc.tile_pool(name="qkp", bufs=1) as qkp,
                tc.tile_pool(name="wqp", bufs=2) as wqp,
                tc.tile_pool(name="wpp", bufs=2) as wpp,
                tc.tile_pool(name="ptp", bufs=2) as ptp,
                tc.tile_pool(name="lnt", bufs=2) as lnt,
                tc.tile_pool(name="psB", bufs=1, space="PSUM") as psB,
                tc.tile_pool(name="psC", bufs=1, space="PSUM") as psC,
                tc.tile_pool(name="psD", bufs=1, space="PSUM") as psD,
            ):
                # A: load pre-transposed x directly (host supplies xT)
                xT = [xtp.tile([128, ROWS], BF16, tag=f"x{kk}", name=f"x{kk}")
                      for kk in range(KT)]
                for c0, c1 in [(0, 512), (512, 1568), (1568, 3136), (3136, 4704), (4704, ROWS)]:
                    for kk in range(KT):
                        nc.sync.dma_start(xT[kk][:, c0:c1], xd[kk][:, c0:c1])

                ei = 0
                sS = []
                gds = []

                def emit_pass2(c2, bb, pspool, tagT, tagR, sbp):
                    bsl = slice(bb * 64, (bb + 1) * 64)
                    idx = c2 * 2 + bb
                    rel = sbp.tile([128, 64], F32, tag="rel", name="rel")
                    src = gds[idx][:, :].copy()
                    src.ap = _br.VecI64Pair([[64 * 127, 2], [126, 64], [1, 64]])
                    src.offset = src.offset + 63
                    nc.sync.dma_start(rel[:, :], src)
                    s2 = sbp.tile([128, 64], F32, tag="s2", name="s2")
                    nc.vector.tensor_add(s2[:], sS[idx][:], rel[:])
                    ex = sbp.tile([128, 64], F32, tag="ex", name="ex")
                    den = sbp.tile([128, 1], F32, tag="den", name="den")
                    nc.scalar.activation(ex[:], s2[:], AF.Exp,
                                         bias=0.0, scale=1.0, accum_out=den[:])
                    rden = sbp.tile([128, 1], F32, tag="rden", name="rden")
                    nc.vector.reciprocal(rden[:], den[:])
                    exn = sbp.tile([128, 64], F32, tag="exn", name="exn")
                    nc.vector.tensor_scalar_mul(exn[:], ex[:], rden[:])
                    pTT = pspool.tile([128, 64], F32, tag=tagT, name="pTT")
                    for h in range(2):
                        hsl = slice(h * 64, (h + 1) * 64)
                        pTr = pspool.tile([64, 64], F32, tag=tagR, name="pTr")
                        nc.tensor.transpose(pTr[:], exn[hsl, :], c_id[hsl, hsl])
                        aTh = sbp.tile([64, 64], F32, tag="aT", name="aTh")
                        nc.vector.tensor_copy(aTh[:], pTr[:])
                        co = (h * 2 + bb) * 64
                        nc.tensor.matmul(pTT[hsl, :], vre[c2][:, co:co + 64],
                                         aTh[:], start=True, stop=True)
                    nc.vector.tensor_add(outT[c2][:, bsl], pTT[:, :],
                                         qT[c2][:, bsl])

                for sg in range(6):
                    j = sg // 2
                    # B: qkv projection for 3 chunks, group-major so all
                    # chunks consume freshly arrived xT columns
                    qks = [qkp.tile([128, ROWS], BF16, tag=f"qk{i}", name=f"qk{i}")
                           for i in range(3)]
                    wqas = []
                    for i in range(3):
                        wqa = wqp.tile([128, D], BF16, tag=f"wqa{i}",
                                       name=f"wqa{i}")
                        nc.gpsimd.dma_start(wqa[:], Wqc[sg * 3 + i])
                        wqas.append(wqa)
                    for grp in G3:
                        for i in range(3):
                            pts = []
                            for t, (goff, gsz) in enumerate(grp):
                                pts.append(psB.tile([128, gsz], F32,
                                                    tag=f"pb{t}", name=f"pb{t}"))
                            for t, (goff, gsz) in enumerate(grp):
                                for kk in range(KT):
                                    nc.tensor.matmul(
                                        pts[t][:], wqas[i][:, kk * 128:(kk + 1) * 128],
                                        xT[kk][:, goff:goff + gsz],
                                        start=(kk == 0), stop=(kk == KT - 1))
                            for t, (goff, gsz) in enumerate(grp):
                                if ei % 2 == 0:
                                    nc.vector.tensor_copy(
                                        qks[i][:, goff:goff + gsz], pts[t][:])
                                else:
                                    nc.scalar.copy(
                                        qks[i][:, goff:goff + gsz], pts[t][:])
                                ei += 1
                    # C: pooling conv, accumulate over hw
                    pas = [psC.tile([128, 128], F32, tag=f"pa{i}", name=f"pa{i}")
                           for i in range(3)]
                    if sg % 2 == 0:
                        wpa = wpp.tile([128, 3200], BF16, tag="wpa")
                        nc.gpsimd.dma_start(wpa[:], Wpb[j][:, 0:3200])
                        wpb2 = wpp.tile([128, 3072], BF16, tag="wpb2")
                        nc.gpsimd.dma_start(wpb2[:], Wpb[j][:, 3200:6272])
                        wpcache[j] = (wpa, wpb2)
                    else:
                        wpa, wpb2 = wpcache[j]
                    for hw in range(HW2):
                        if hw < 25:
                            wp = wpa[:, hw * 128:(hw + 1) * 128]
                        else:
                            wp = wpb2[:, (hw - 25) * 128:(hw - 24) * 128]
                        for i in range(3):
                            nc.tensor.matmul(
                                pas[i][:], wp,
                                qks[i][:, hw:ROWS:HW2],
                                start=(hw == 0), stop=(hw == HW2 - 1))
                    # D: bias + layernorm per chunk, inline
                    for i in range(3):
                        ch = sg * 3 + i
                        i6 = ch % 6
                        pt0 = ptp.tile([128, 128], F32, tag="pt0")
                        nc.vector.tensor_scalar_add(pt0[:], pas[i][:],
                                                    c_pb[:, j:j + 1])
                        p1 = psD.tile([128, 128], F32, tag="ptr")
                        nc.tensor.transpose(p1[:], pt0[:], c_id[:])
                        row = lnt.tile([128, 128], F32, tag="row")
                        if ch % 2 == 0:
                            nc.vector.tensor_copy(row[:], p1[:])
                        else:
                            nc.scalar.copy(row[:], p1[:])
                        rs = lnt.tile([128, 2], F32, tag="rs")
                        for h in range(2):
                            nc.vector.tensor_reduce(
                                rs[:, h:h + 1], row[:, h * 64:(h + 1) * 64],
                                axis=AX.X, op=OP.add)
                        mean = lnt.tile([128, 2], F32, tag="mean")
                        nc.vector.tensor_scalar_mul(mean[:], rs[:], 1.0 / 64)
                        cen = lnt.tile([128, 128], F32, tag="cen")
                        for h in range(2):
                            nc.vector.tensor_scalar_sub(
                                cen[:, h * 64:(h + 1) * 64],
                                row[:, h * 64:(h + 1) * 64], mean[:, h:h + 1])
                        sq = lnt.tile([128, 128], F32, tag="sq")
                        nc.vector.tensor_mul(sq[:], cen[:], cen[:])
                        vs = lnt.tile([128, 2], F32, tag="vs")
                        for h in range(2):
                            nc.vector.tensor_reduce(
                                vs[:, h:h + 1], sq[:, h * 64:(h + 1) * 64],
                                axis=AX.X, op=OP.add)
                        std = lnt.tile([128, 2], F32, tag="std")
                        nc.scalar.activation(std[:], vs[:], AF.Sqrt,
                                             bias=c_eps[:], scale=1.0 / 64)
                        rstd = lnt.tile([128, 2], F32, tag="rstd")
                        nc.vector.reciprocal(rstd[:], std[:])
                        nrm = lnt.tile([128, 128], F32, tag="nrm")
                        for h in range(2):
                            nc.vector.tensor_scalar_mul(
                                nrm[:, h * 64:(h + 1) * 64],
                                cen[:, h * 64:(h + 1) * 64], rstd[:, h:h + 1])
                        tmp = lnt.tile([128, 128], F32, tag="tmp")
                        nc.vector.tensor_mul(tmp[:], nrm[:],
                                             c_ag[:, j * 128:(j + 1) * 128])
                        if j == 2:
                            nc.vector.tensor_add(vrow[i6][:], tmp[:],
                                                 c_ab[:, j * 128:(j + 1) * 128])
                            for h in range(2):
                                for bb in range(2):
                                    co = (h * 2 + bb) * 64
                                    nc.sync.dma_start(
                                        vre[i6][:, co:co + 64],
                                        vrow[i6][bb * 64:(bb + 1) * 64,
                                                 h * 64:(h + 1) * 64])
                        else:
                            fin = lnt.tile([128, 128], F32, tag="fin")
                            nc.vector.tensor_add(fin[:], tmp[:],
                                                 c_ab[:, j * 128:(j + 1) * 128])
                            p2 = psD.tile([128, 128], F32, tag="ptr2")
                            nc.tensor.transpose(p2[:], fin[:], c_id[:])
                            dst = qT[i6] if j == 0 else kTt[i6]
                            if ch % 2 == 0:
                                nc.vector.tensor_copy(dst[:], p2[:])
                            else:
                                nc.scalar.copy(dst[:], p2[:])
                    if sg == 4:
                        for c2e in range(3):
                            for bbe in range(2):
                                emit_pass2(c2e, bbe, psD, "ptr2", "ptr", atp)
                    if sg == 3:
                        # attention pass 1: S = q.k, G = q.rpt (needs only q, k)
                        for c2 in range(6):
                            for bb in range(2):
                                bsl = slice(bb * 64, (bb + 1) * 64)
                                pSG = psD.tile([128, 192], F32, tag="ptr2", name="pSG")
                                for h in range(2):
                                    hsl = slice(h * 64, (h + 1) * 64)
                                    nc.tensor.matmul(pSG[hsl, 0:64],
                                                     qT[c2][hsl, bsl],
                                                     kTt[c2][hsl, bsl],
                                                     start=True, stop=True)
                                    nc.tensor.matmul(pSG[hsl, 64:191],
                                                     qT[c2][hsl, bsl],
                                                     c_rpt[hsl, :],
                                                     start=True, stop=True)
                                sSt = atp.tile([128, 64], F32, tag=f"sS{c2}_{bb}",
                                               name=f"sS{c2}_{bb}", bufs=1)
                                nc.vector.tensor_copy(sSt[:], pSG[:, 0:64])
                                sS.append(sSt)
                                gsb = atp.tile([128, 127], F32, tag="gsb")
                                nc.scalar.copy(gsb[:], pSG[:, 64:191])
                                gd = drp.tile([128, 127], F32, tag="gd")
                                nc.sync.dma_start(gd[:], gsb[:])
                                gds.append(gd)

            # ---------------- stage E: attention pass 2 + projection ----------------
            with (
                tc.tile_pool(name="psO", bufs=3, space="PSUM") as psO,
                tc.tile_pool(name="att", bufs=6) as att,
                tc.tile_pool(name="prj", bufs=1) as prj,
                tc.tile_pool(name="psY", bufs=1, space="PSUM") as psY,
            ):
                c_wpr = []
                for cc in range(KT):
                    w = prj.tile([128, D], F32, tag=f"wpr{cc}", name=f"wpr{cc}")
                    nc.gpsimd.dma_start(w[:], Wpr[cc])
                    c_wpr.append(w)
                c_bpr = prj.tile([128, D], F32, tag="bpr")
                nc.gpsimd.dma_start(c_bpr[:], bprojR[:, :])
                ysb = prj.tile([128, D], F32, tag="ysb")
                for c2 in range(3, 6):
                    for bb in range(2):
                        emit_pass2(c2, bb, psO, "pTT", "pTr", att)

                # projection (accumulates per chunk as outT completes)
                for goff, gsz in [(0, 512), (512, 256)]:
                    pY = psY.tile([128, gsz], F32, tag="pY")
                    for cc in range(KT):
                        nc.tensor.matmul(pY[:], outT[cc][:],
                                         c_wpr[cc][:, goff:goff + gsz],
                                         start=(cc == 0), stop=(cc == KT - 1))
                    nc.vector.tensor_add(ysb[:, goff:goff + gsz], pY[:],
                                         c_bpr[:, goff:goff + gsz])
                    nc.sync.dma_start(yout[:, goff:goff + gsz],
                                      ysb[:, goff:goff + gsz])
            atp.release()

    nc.compile()
    return nc


def _host_prep(W_qkv, Wpq, bpq, Wpk, bpk, Wpv, bpv,
               g_q, be_q, g_k, be_k, g_v, be_v, rel_pos_t, W_proj, b_proj):
    bf = ml_dtypes.bfloat16
    Wqc = np.ascontiguousarray(
        np.asarray(W_qkv, np.float32).reshape(KT, 128, NCH, 128)
        .transpose(2, 1, 0, 3).reshape(NCH, 128, D)).astype(bf)
    Wpb = np.zeros((3, HW2, 128, 128), np.float32)
    for j, Wp in enumerate((Wpq, Wpk, Wpv)):
        Wp = np.asarray(Wp, np.float32)                      # (dout, cin, 7, 7)
        WpT = Wp.transpose(2, 3, 1, 0).reshape(HW2, C, C)    # (hw, ci, dout)
        Wpb[j, :, 0:64, 0:64] = WpT
        Wpb[j, :, 64:128, 64:128] = WpT
    Wpb = np.ascontiguousarray(
        Wpb.transpose(0, 2, 1, 3).reshape(3, 128, HW2 * 128)).astype(bf)
    rp = np.ascontiguousarray(np.asarray(rel_pos_t, np.float32)[::-1].T)  # (64,127)
    Rpt = np.concatenate([rp, rp], axis=0)                   # (128,127)

    gq, gk, gv = (np.asarray(a, np.float32) for a in (g_q, g_k, g_v))
    bq, bk, bv = (np.asarray(a, np.float32) for a in (be_q, be_k, be_v))
    gk = gk * SCALE
    bk = bk * SCALE
    affG = np.broadcast_to(
        np.concatenate([np.tile(g, 2) for g in (gq, gk, gv)])[None, :],
        (128, 384)).copy()
    affB = np.broadcast_to(
        np.concatenate([np.tile(b, 2) for b in (bq, bk, bv)])[None, :],
        (128, 384)).copy()
    pbias = np.ascontiguousarray(np.stack(
        [np.tile(np.asarray(b, np.float32), 2) for b in (bpq, bpk, bpv)], axis=1))
    Wpr = np.ascontiguousarray(
        np.asarray(W_proj, np.float32).reshape(KT, 128, D))
    bprojR = np.broadcast_to(
        np.asarray(b_proj, np.float32)[None, :], (128, D)).copy()
    idtf = np.eye(128, dtype=np.float32)
    return {"Wqc": Wqc, "Wpb": Wpb, "Rpt": Rpt, "affG": affG, "affB": affB,
            "pbias": pbias, "Wpr": Wpr, "bprojR": bprojR, "idtf": idtf}


def kernel(x, W_qkv, Wpq, bpq, Wpk, bpk, Wpv, bpv,
           g_q, be_q, g_k, be_k, g_v, be_v, rel_pos_t, W_proj, b_proj):
    global LAST_RESULT
    if "nc" not in _cache:
        _cache["nc"] = _build_nc()
    nc = _cache["nc"]

    shared = _host_prep(W_qkv, Wpq, bpq, Wpk, bpk, Wpv, bpv,
                        g_q, be_q, g_k, be_k, g_v, be_v,
                        rel_pos_t, W_proj, b_proj)
    bf = ml_dtypes.bfloat16
    xr = np.asarray(x, np.float32).reshape(B, T, HW2, D)
    in_maps = []
    for i in range(NCORES):
        xc = xr[i * BLOC:(i + 1) * BLOC].reshape(ROWS, D).astype(bf)
        xs = np.ascontiguousarray(xc.T).reshape(KT, 128, ROWS)
        m = {"xd": xs}
        m.update(shared)
        in_maps.append(m)

    res = run_bass_kernel_spmd(nc, in_maps, core_ids=list(range(NCORES)))
    LAST_RESULT = res
    y = np.stack([np.asarray(res.results[i]["y"], np.float32)
                  for i in range(NCORES)], axis=0)      # (8, 128, 768)
    return y.reshape(B, T, D).astype(np.float32)


# revision 4
# speedup vs baseline: 1.0539x; 1.0002x over previous
import numpy as np
import ml_dtypes
import bass_rust as _br

import concourse.bass as bass
import concourse.mybir as mybir
import concourse.tile as tile
from concourse import bacc
from concourse.bass_utils import run_bass_kernel_spmd
from concourse.tile_rust import add_dep_helper

F32 = mybir.dt.float32
BF16 = mybir.dt.bfloat16
AX = mybir.AxisListType
AF = mybir.ActivationFunctionType
OP = mybir.AluOpType

B, T, HWs, D, N, C = 16, 64, 7, 768, 12, 64
HW2 = HWs * HWs                 # 49
NCORES = 8
BLOC = B // NCORES              # 2
ROWS = BLOC * T * HW2           # 6272
NCOL = 3 * D                    # 2304
NCH = NCOL // 128               # 18 qkvT chunks
KT = D // 128                   # 6
EPS = 1e-5
SCALE = C ** -0.5
CGRPS = [(g * 512, 512) for g in range(12)] + [(6144, 128)]
G3 = [CGRPS[0:3], CGRPS[3:6], CGRPS[6:9], CGRPS[9:12], CGRPS[12:13]]
XCHUNKS = [(0, 512), (512, 1536), (2048, 2048), (4096, 2176)]

_cache = {}
LAST_RESULT = None


def _build_nc():
    nc = bacc.Bacc(None, target_bir_lowering=False, debug=False)
    xd = nc.declare_dram_parameter("xd", [KT, 128, ROWS], BF16, isOutput=False)
    Wqc = nc.declare_dram_parameter("Wqc", [NCH, 128, D], BF16, isOutput=False)
    Wpb = nc.declare_dram_parameter("Wpb", [3, 128, HW2 * 128], BF16, isOutput=False)
    Rpt = nc.declare_dram_parameter("Rpt", [128, 127], F32, isOutput=False)
    affG = nc.declare_dram_parameter("affG", [128, 384], F32, isOutput=False)
    affB = nc.declare_dram_parameter("affB", [128, 384], F32, isOutput=False)
    pbias = nc.declare_dram_parameter("pbias", [128, 3], F32, isOutput=False)
    Wpr = nc.declare_dram_parameter("Wpr", [KT, 128, D], F32, isOutput=False)
    bprojR = nc.declare_dram_parameter("bprojR", [128, D], F32, isOutput=False)
    idtf = nc.declare_dram_parameter("idtf", [128, 128], F32, isOutput=False)
    yout = nc.declare_dram_parameter("y", [128, D], F32, isOutput=True)

    wpcache = {}
    with tile.TileContext(nc) as tc:
        with (
            tc.tile_pool(name="cst", bufs=1) as cst,
            tc.tile_pool(name="qkv", bufs=1) as qkv,
            tc.tile_pool(name="drp", bufs=12, space="DRAM") as drp,
        ):
            c_rpt = cst.tile([128, 127], F32, tag="rpt")
            nc.scalar.dma_start(c_rpt[:], Rpt[:, :])
            c_ag = cst.tile([128, 384], F32, tag="ag")
            nc.scalar.dma_start(c_ag[:], affG[:, :])
            c_ab = cst.tile([128, 384], F32, tag="ab")
            nc.scalar.dma_start(c_ab[:], affB[:, :])
            c_pb = cst.tile([128, 3], F32, tag="pb")
            nc.scalar.dma_start(c_pb[:], pbias[:, :])
            c_id = cst.tile([128, 128], F32, tag="idf")
            nc.scalar.dma_start(c_id[:], idtf[:, :])
            c_eps = cst.tile([128, 1], F32, tag="eps")
            nc.vector.memset(c_eps[:], EPS)

            atp = tc.alloc_tile_pool(name="atp", bufs=6)
            qT = [qkv.tile([128, 128], F32, tag=f"q{i}", name=f"q{i}") for i in range(6)]
            kTt = [qkv.tile([128, 128], F32, tag=f"k{i}", name=f"k{i}") for i in range(6)]
            vrow = [qkv.tile([128, 128], F32, tag=f"v{i}", name=f"v{i}") for i in range(6)]
            vre = [qkv.tile([64, 256], F32, tag=f"vr{i}", name=f"vr{i}") for i in range(6)]
            outT = [qkv.tile([128, 128], F32, tag=f"o{i}", name=f"o{i}") for i in range(6)]

            # ---------------- stages A/B/C/D fused ----------------
            with (
                tc.tile_pool(name="xtp", bufs=1) as xtp,
                tc.tile_pool(name="qkp", bufs=1) as qkp,
                tc.tile_pool(name="wqp", bufs=2) as wqp,
                tc.tile_pool(name="wpp", bufs=2) as wpp,
                tc.tile_pool(name="ptp", bufs=2) as ptp,
                tc.tile_pool(name="lnt", bufs=2) as lnt,
                tc.tile_pool(name="psB", bufs=1, space="PSUM") as psB,
                tc.tile_pool(name="psC", bufs=1, space="PSUM") as psC,
                tc.tile_pool(name="psD", bufs=1, space="PSUM") as psD,
            ):
                # A: load pre-transposed x directly (host supplies xT)
                xT = [xtp.tile([128, ROWS], BF16, tag=f"x{kk}", name=f"x{kk}")
                      for kk in range(KT)]
                for c0, c1 in [(0, 512), (512, 1568), (1568, 3136), (3136, 4704), (4704, ROWS)]:
                    for kk in range(KT):
                        nc.sync.dma_start(xT[kk][:, c0:c1], xd[kk][:, c0:c1])

                ei = 0
                sS = []
                gds = []

                def emit_pass2(c2, bb, pspool, tagT, tagR, sbp):
                    bsl = slice(bb * 64, (bb + 1) * 64)
                    idx = c2 * 2 + bb
                    rel = sbp.tile([128, 64], F32, tag="rel", name="rel")
                    src = gds[idx][:, :].copy()
                    src.ap = _br.VecI64Pair([[64 * 127, 2], [126, 64], [1, 64]])
                    src.offset = src.offset + 63
                    nc.sync.dma_start(rel[:, :], src)
                    s2 = sbp.tile([128, 64], F32, tag="s2", name="s2")
                    nc.vector.tensor_add(s2[:], sS[idx][:], rel[:])
                    ex = sbp.tile([128, 64], F32, tag="ex", name="ex")
                    den = sbp.tile([128, 1], F32, tag="den", name="den")
                    nc.scalar.activation(ex[:], s2[:], AF.Exp,
                                         bias=0.0, scale=1.0, accum_out=den[:])
                    rden = sbp.tile([128, 1], F32, tag="rden", name="rden")
                    nc.vector.reciprocal(rden[:], den[:])
                    exn = sbp.tile([128, 64], F32, tag="exn", name="exn")
                    nc.vector.tensor_scalar_mul(exn[:], ex[:], rden[:])
                    pTT = pspool.tile([128, 64], F32, tag=tagT, name="pTT")
                    for h in range(2):
                        hsl = slice(h * 64, (h + 1) * 64)
                        pTr = pspool.tile([64, 64], F32, tag=tagR, name="pTr")
                        nc.tensor.transpose(pTr[:], exn[hsl, :], c_id[hsl, hsl])
                        aTh = sbp.tile([64, 64], F32, tag="aT", name="aTh")
                        nc.vector.tensor_copy(aTh[:], pTr[:])
                        co = (h * 2 + bb) * 64
                        nc.tensor.matmul(pTT[hsl, :], vre[c2][:, co:co + 64],
                                         aTh[:], start=True, stop=True)
                    nc.vector.tensor_add(outT[c2][:, bsl], pTT[:, :],
                                         qT[c2][:, bsl])

                for sg in range(6):
                    j = sg // 2
                    # B: qkv projection for 3 chunks, group-major so all
                    # chunks consume freshly arrived xT columns
                    qks = [qkp.tile([128, ROWS], BF16, tag=f"qk{i}", name=f"qk{i}")
                           for i in range(3)]
                    wqas = []
                    for i in range(3):
                        wqa = wqp.tile([128, D], BF16, tag=f"wqa{i}",
                                       name=f"wqa{i}")
                        nc.gpsimd.dma_start(wqa[:], Wqc[sg * 3 + i])
                        wqas.append(wqa)
                    for grp in G3:
                        for i in range(3):
                            pts = []
                            for t, (goff, gsz) in enumerate(grp):
                                pts.append(psB.tile([128, gsz], F32,
                                                    tag=f"pb{t}", name=f"pb{t}"))
                            for t, (goff, gsz) in enumerate(grp):
                                for kk in range(KT):
                                    nc.tensor.matmul(
                                        pts[t][:], wqas[i][:, kk * 128:(kk + 1) * 128],
                                        xT[kk][:, goff:goff + gsz],
                                        start=(kk == 0), stop=(kk == KT - 1))
                            for t, (goff, gsz) in enumerate(grp):
                                if ei % 2 == 0:
                                    nc.vector.tensor_copy(
                                        qks[i][:, goff:goff + gsz], pts[t][:])
                                else:
                                    nc.scalar.copy(
                                        qks[i][:, goff:goff + gsz], pts[t][:])
                                ei += 1
                    # C: pooling conv, accumulate over hw
                    pas = [psC.tile([128, 128], F32, tag=f"pa{i}", name=f"pa{i}")
                           for i in range(3)]
                    if sg % 2 == 0:
                        wpa = wpp.tile([128, 3200], BF16, tag="wpa")
                        nc.gpsimd.dma_start(wpa[:], Wpb[j][:, 0:3200])
                        wpb2 = wpp.tile([128, 3072], BF16, tag="wpb2")
                        nc.gpsimd.dma_start(wpb2[:], Wpb[j][:, 3200:6272])
                        wpcache[j] = (wpa, wpb2)
                    else:
                        wpa, wpb2 = wpcache[j]
                    for hw in range(HW2):
                        if hw < 25:
                            wp = wpa[:, hw * 128:(hw + 1) * 128]
                        else:
                            wp = wpb2[:, (hw - 25) * 128:(hw - 24) * 128]
                        for i in range(3):
                            nc.tensor.matmul(
                                pas[i][:], wp,
                                qks[i][:, hw:ROWS:HW2],
                                start=(hw == 0), stop=(hw == HW2 - 1))
                    # D: bias + layernorm per chunk, inline
                    for i in range(3):
                        ch = sg * 3 + i
                        i6 = ch % 6
                        pt0 = ptp.tile([128, 128], F32, tag="pt0")
                        nc.vector.tensor_scalar_add(pt0[:], pas[i][:],
                                                    c_pb[:, j:j + 1])
                        p1 = psD.tile([128, 128], F32, tag="ptr")
                        nc.tensor.transpose(p1[:], pt0[:], c_id[:])
                        row = lnt.tile([128, 128], F32, tag="row")
                        if ch % 2 == 0:
                            nc.vector.tensor_copy(row[:], p1[:])
                        else:
                            nc.scalar.copy(row[:], p1[:])
                        rs = lnt.tile([128, 2], F32, tag="rs")
                        for h in range(2):
                            nc.vector.tensor_reduce(
                                rs[:, h:h + 1], row[:, h * 64:(h + 1) * 64],
                                axis=AX.X, op=OP.add)
                        mean = lnt.tile([128, 2], F32, tag="mean")
                        nc.vector.tensor_scalar_mul(mean[:], rs[:], 1.0 / 64)
                        cen = lnt.tile([128, 128], F32, tag="cen")
                        for h in range(2):
                            nc.vector.tensor_scalar_sub(
                                cen[:, h * 64:(h + 1) * 64],
                                row[:, h * 64:(h + 1) * 64], mean[:, h:h + 1])
                        sq = lnt.tile([128, 128], F32, tag="sq")
                        nc.vector.tensor_mul(sq[:], cen[:], cen[:])
                        vs = lnt.tile([128, 2], F32, tag="vs")
                        for h in range(2):
                            nc.vector.tensor_reduce(
                                vs[:, h:h + 1], sq[:, h * 64:(h + 1) * 64],
                                axis=AX.X, op=OP.add)
                        std = lnt.tile([128, 2], F32, tag="std")
                        nc.scalar.activation(std[:], vs[:], AF.Sqrt,
                                             bias=c_eps[:], scale=1.0 / 64)
                        rstd = lnt.tile([128, 2], F32, tag="rstd")
                        nc.vector.reciprocal(rstd[:], std[:])
                        nrm = lnt.tile([128, 128], F32, tag="nrm")
                        for h in range(2):
                            nc.vector.tensor_scalar_mul(
                                nrm[:, h * 64:(h + 1) * 64],
                                cen[:, h * 64:(h + 1) * 64], rstd[:, h:h + 1])
                        tmp = lnt.tile([128, 128], F32, tag="tmp")
                        nc.vector.tensor_mul(tmp[:], nrm[:],
                                             c_ag[:, j * 128:(j + 1) * 128])
                        if j == 2:
                            nc.vector.tensor_add(vrow[i6][:], tmp[:],
                                                 c_ab[:, j * 128:(j + 1) * 128])
                            for h in range(2):
                                for bb in range(2):
                                    co = (h * 2 + bb) * 64
                                    nc.sync.dma_start(
                                        vre[i6][:, co:co + 64],
                                        vrow[i6][bb * 64:(bb + 1) * 64,
                                                 h * 64:(h + 1) * 64])
                        else:
                            fin = lnt.tile([128, 128], F32, tag="fin")
                            nc.vector.tensor_add(fin[:], tmp[:],
                                                 c_ab[:, j * 128:(j + 1) * 128])
                            p2 = psD.tile([128, 128], F32, tag="ptr2")
                            nc.tensor.transpose(p2[:], fin[:], c_id[:])
                            dst = qT[i6] if j == 0 else kTt[i6]
                            if ch % 2 == 0:
                                nc.vector.tensor_copy(dst[:], p2[:])
                            else:
                                nc.scalar.copy(dst[:], p2[:])
                    if sg == 4:
                        for c2e in range(3):
                            for bbe in range(2):
                                emit_pass2(c2e, bbe, psD, "ptr2", "ptr", atp)
                    if sg == 3:
                        # attention pass 1: S = q.k, G = q.rpt (needs only q, k)
                        for c2 in range(6):
                            for bb in range(2):
                                bsl = slice(bb * 64, (bb + 1) * 64)
                                pSG = psD.tile([128, 192], F32, tag="ptr2", name="pSG")
                                for h in range(2):
                                    hsl = slice(h * 64, (h + 1) * 64)
                                    nc.tensor.matmul(pSG[hsl, 0:64],
                                                     qT[c2][hsl, bsl],
                                                     kTt[c2][hsl, bsl],
                                                     start=True, stop=True)
                                    nc.tensor.matmul(pSG[hsl, 64:191],
                                                     qT[c2][hsl, bsl],
                                                     c_rpt[hsl, :],
                                                     start=True, stop=True)
                                sSt = atp.tile([128, 64], F32, tag=f"sS{c2}_{bb}",
                                               name=f"sS{c2}_{bb}", bufs=1)
                                nc.vector.tensor_copy(sSt[:], pSG[:, 0:64])
                                sS.append(sSt)
                                gsb = atp.tile([128, 127], F32, tag="gsb")
                                nc.scalar.copy(gsb[:], pSG[:, 64:191])
                                gd = drp.tile([128, 127], F32, tag="gd")
                                nc.sync.dma_start(gd[:], gsb[:])
                                gds.append(gd)

            # ---------------- stage E: attention pass 2 + projection ----------------
            with (
                tc.tile_pool(name="psO", bufs=3, space="PSUM") as psO,
                tc.tile_pool(name="att", bufs=8) as att,
                tc.tile_pool(name="prj", bufs=1) as prj,
                tc.tile_pool(name="psY", bufs=1, space="PSUM") as psY,
            ):
                c_wpr = []
                for cc in range(KT):
                    w = prj.tile([128, D], F32, tag=f"wpr{cc}", name=f"wpr{cc}")
                    nc.gpsimd.dma_start(w[:], Wpr[cc])
                    c_wpr.append(w)
                c_bpr = prj.tile([128, D], F32, tag="bpr")
                nc.gpsimd.dma_start(c_bpr[:], bprojR[:, :])
                ysb = prj.tile([128, D], F32, tag="ysb")
                for c2 in range(3, 6):
                    for bb in range(2):
                        emit_pass2(c2, bb, psO, "pTT", "pTr", att)

                # projection (accumulates per chunk as outT completes)
                for goff, gsz in [(0, 512), (512, 256)]:
                    pY = psY.tile([128, gsz], F32, tag="pY")
                    for cc in range(KT):
                        nc.tensor.matmul(pY[:], outT[cc][:],
                                         c_wpr[cc][:, goff:goff + gsz],
                                         start=(cc == 0), stop=(cc == KT - 1))
                    nc.vector.tensor_add(ysb[:, goff:goff + gsz], pY[:],
                                         c_bpr[:, goff:goff + gsz])
                    nc.sync.dma_start(yout[:, goff:goff + gsz],
                                      ysb[:, goff:goff + gsz])
            atp.release()

    nc.compile()
    return nc


def _host_prep(W_qkv, Wpq, bpq, Wpk, bpk, Wpv, bpv,
               g_q, be_q, g_k, be_k, g_v, be_v, rel_pos_t, W_proj, b_proj):
    bf = ml_dtypes.bfloat16
    Wqc = np.ascontiguousarray(
        np.asarray(W_qkv, np.float32).reshape(KT, 128, NCH, 128)
        .transpose(2, 1, 0, 3).reshape(NCH, 128, D)).astype(bf)
    Wpb = np.zeros((3, HW2, 128, 128), np.float32)
    for j, Wp in enumerate((Wpq, Wpk, Wpv)):
        Wp = np.asarray(Wp, np.float32)                      # (dout, cin, 7, 7)
        WpT = Wp.transpose(2, 3, 1, 0).reshape(HW2, C, C)    # (hw, ci, dout)
        Wpb[j, :, 0:64, 0:64] = WpT
        Wpb[j, :, 64:128, 64:128] = WpT
    Wpb = np.ascontiguousarray(
        Wpb.transpose(0, 2, 1, 3).reshape(3, 128, HW2 * 128)).astype(bf)
    rp = np.ascontiguousarray(np.asarray(rel_pos_t, np.float32)[::-1].T)  # (64,127)
    Rpt = np.concatenate([rp, rp], axis=0)                   # (128,127)

    gq, gk, gv = (np.asarray(a, np.float32) for a in (g_q, g_k, g_v))
    bq, bk, bv = (np.asarray(a, np.float32) for a in (be_q, be_k, be_v))
    gk = gk * SCALE
    bk = bk * SCALE
    affG = np.broadcast_to(
        np.concatenate([np.tile(g, 2) for g in (gq, gk, gv)])[None, :],
        (128, 384)).copy()
    affB = np.broadcast_to(
        np.concatenate([np.tile(b, 2) for b in (bq, bk, bv)])[None, :],
        (128, 384)).copy()
    pbias = np.ascontiguousarray(np.stack(
        [np.tile(np.asarray(b, np.float32), 2) for b in (bpq, bpk, bpv)], axis=1))
    Wpr = np.ascontiguousarray(
        np.asarray(W_proj, np.float32).reshape(KT, 128, D))
    bprojR = np.broadcast_to(
        np.asarray(b_proj, np.float32)[None, :], (128, D)).copy()
    idtf = np.eye(128, dtype=np.float32)
    return {"Wqc": Wqc, "Wpb": Wpb, "Rpt": Rpt, "affG": affG, "affB": affB,
            "pbias": pbias, "Wpr": Wpr, "bprojR": bprojR, "idtf": idtf}


def kernel(x, W_qkv, Wpq, bpq, Wpk, bpk, Wpv, bpv,
           g_q, be_q, g_k, be_k, g_v, be_v, rel_pos_t, W_proj, b_proj):
    global LAST_RESULT
    if "nc" not in _cache:
        _cache["nc"] = _build_nc()
    nc = _cache["nc"]

    shared = _host_prep(W_qkv, Wpq, bpq, Wpk, bpk, Wpv, bpv,
                        g_q, be_q, g_k, be_k, g_v, be_v,
                        rel_pos_t, W_proj, b_proj)
    bf = ml_dtypes.bfloat16
    xr = np.asarray(x, np.float32).reshape(B, T, HW2, D)
    in_maps = []
    for i in range(NCORES):
        xc = xr[i * BLOC:(i + 1) * BLOC].reshape(ROWS, D).astype(bf)
        xs = np.ascontiguousarray(xc.T).reshape(KT, 128, ROWS)
        m = {"xd": xs}
        m.update(shared)
        in_maps.append(m)

    res = run_bass_kernel_spmd(nc, in_maps, core_ids=list(range(NCORES)))
    LAST_RESULT = res
    y = np.stack([np.asarray(res.results[i]["y"], np.float32)
                  for i in range(NCORES)], axis=0)      # (8, 128, 768)
    return y.reshape(B, T, D).astype(np.float32)


# revision 5
# speedup vs baseline: 1.0838x; 1.0284x over previous
import numpy as np
import ml_dtypes
import bass_rust as _br

import concourse.bass as bass
import concourse.mybir as mybir
import concourse.tile as tile
from concourse import bacc
from concourse.bass_utils import run_bass_kernel_spmd
from concourse.tile_rust import add_dep_helper

F32 = mybir.dt.float32
BF16 = mybir.dt.bfloat16
AX = mybir.AxisListType
AF = mybir.ActivationFunctionType
OP = mybir.AluOpType

B, T, HWs, D, N, C = 16, 64, 7, 768, 12, 64
HW2 = HWs * HWs                 # 49
NCORES = 8
BLOC = B // NCORES              # 2
ROWS = BLOC * T * HW2           # 6272
NCOL = 3 * D                    # 2304
NCH = NCOL // 128               # 18 qkvT chunks
KT = D // 128                   # 6
EPS = 1e-5
SCALE = C ** -0.5
CGRPS = [(g * 512, 512) for g in range(12)] + [(6144, 128)]
G3 = [CGRPS[0:3], CGRPS[3:6], CGRPS[6:9], CGRPS[9:12], CGRPS[12:13]]
XCHUNKS = [(0, 512), (512, 1536), (2048, 2048), (4096, 2176)]

_cache = {}
LAST_RESULT = None


def _build_nc():
    nc = bacc.Bacc(None, target_bir_lowering=False, debug=False)
    xd = nc.declare_dram_parameter("xd", [KT, 128, ROWS], BF16, isOutput=False)
    Wqc = nc.declare_dram_parameter("Wqc", [NCH, 128, D], BF16, isOutput=False)
    Wpb = nc.declare_dram_parameter("Wpb", [3, 128, HW2 * 128], BF16, isOutput=False)
    Rpt = nc.declare_dram_parameter("Rpt", [128, 127], BF16, isOutput=False)
    affG = nc.declare_dram_parameter("affG", [128, 384], F32, isOutput=False)
    affB = nc.declare_dram_parameter("affB", [128, 384], F32, isOutput=False)
    pbias = nc.declare_dram_parameter("pbias", [128, 3], F32, isOutput=False)
    Wpr = nc.declare_dram_parameter("Wpr", [KT, 128, D], BF16, isOutput=False)
    bprojR = nc.declare_dram_parameter("bprojR", [128, D], F32, isOutput=False)
    idtf = nc.declare_dram_parameter("idtf", [128, 128], F32, isOutput=False)
    idtb = nc.declare_dram_parameter("idtb", [128, 128], BF16, isOutput=False)
    yout = nc.declare_dram_parameter("y", [128, D], F32, isOutput=True)

    wpcache = {}
    with tile.TileContext(nc) as tc:
        with (
            tc.tile_pool(name="cst", bufs=1) as cst,
            tc.tile_pool(name="qkv", bufs=1) as qkv,
            tc.tile_pool(name="drp", bufs=12, space="DRAM") as drp,
        ):
            c_rpt = cst.tile([128, 127], BF16, tag="rpt")
            nc.scalar.dma_start(c_rpt[:], Rpt[:, :])
            c_ag = cst.tile([128, 384], F32, tag="ag")
            nc.scalar.dma_start(c_ag[:], affG[:, :])
            c_ab = cst.tile([128, 384], F32, tag="ab")
            nc.scalar.dma_start(c_ab[:], affB[:, :])
            c_pb = cst.tile([128, 3], F32, tag="pb")
            nc.scalar.dma_start(c_pb[:], pbias[:, :])
            c_id = cst.tile([128, 128], F32, tag="idf")
            nc.scalar.dma_start(c_id[:], idtf[:, :])
            c_eps = cst.tile([128, 1], F32, tag="eps")
            nc.vector.memset(c_eps[:], EPS)
            c_idb = cst.tile([128, 128], BF16, tag="idb")
            nc.scalar.dma_start(c_idb[:], idtb[:, :])

            atp = tc.alloc_tile_pool(name="atp", bufs=6)
            qT = [qkv.tile([128, 128], BF16, tag=f"q{i}", name=f"q{i}") for i in range(6)]
            kTt = [qkv.tile([128, 128], BF16, tag=f"k{i}", name=f"k{i}") for i in range(6)]
            vrow = [qkv.tile([128, 128], BF16, tag=f"v{i}", name=f"v{i}") for i in range(6)]
            vre = [qkv.tile([64, 256], BF16, tag=f"vr{i}", name=f"vr{i}") for i in range(6)]
            outT = [qkv.tile([128, 128], BF16, tag=f"o{i}", name=f"o{i}") for i in range(6)]

            # ---------------- stages A/B/C/D fused ----------------
            with (
                tc.tile_pool(name="xtp", bufs=1) as xtp,
                tc.tile_pool(name="qkp", bufs=1) as qkp,
                tc.tile_pool(name="wqp", bufs=2) as wqp,
                tc.tile_pool(name="wpp", bufs=2) as wpp,
                tc.tile_pool(name="ptp", bufs=2) as ptp,
                tc.tile_pool(name="lnt", bufs=2) as lnt,
                tc.tile_pool(name="psB", bufs=1, space="PSUM") as psB,
                tc.tile_pool(name="psC", bufs=1, space="PSUM") as psC,
                tc.tile_pool(name="psD", bufs=1, space="PSUM") as psD,
            ):
                # A: load pre-transposed x directly (host supplies xT)
                xT = [xtp.tile([128, ROWS], BF16, tag=f"x{kk}", name=f"x{kk}")
                      for kk in range(KT)]
                for c0, c1 in [(0, 512), (512, 1568), (1568, 3136), (3136, 4704), (4704, ROWS)]:
                    for kk in range(KT):
                        nc.sync.dma_start(xT[kk][:, c0:c1], xd[kk][:, c0:c1])

                ei = 0
                sS = []
                gds = []

                def emit_pass2(c2, bb, pspool, tagT, tagR, sbp):
                    bsl = slice(bb * 64, (bb + 1) * 64)
                    idx = c2 * 2 + bb
                    rel = sbp.tile([128, 64], F32, tag="rel", name="rel")
                    src = gds[idx][:, :].copy()
                    src.ap = _br.VecI64Pair([[64 * 127, 2], [126, 64], [1, 64]])
                    src.offset = src.offset + 63
                    nc.sync.dma_start(rel[:, :], src)
                    s2 = sbp.tile([128, 64], F32, tag="s2", name="s2")
                    nc.vector.tensor_add(s2[:], sS[idx][:], rel[:])
                    ex = sbp.tile([128, 64], F32, tag="ex", name="ex")
                    den = sbp.tile([128, 1], F32, tag="den", name="den")
                    nc.scalar.activation(ex[:], s2[:], AF.Exp,
                                         bias=0.0, scale=1.0, accum_out=den[:])
                    rden = sbp.tile([128, 1], F32, tag="rden", name="rden")
                    nc.vector.reciprocal(rden[:], den[:])
                    exn = sbp.tile([128, 64], BF16, tag="exn", name="exn")
                    nc.vector.tensor_scalar_mul(exn[:], ex[:], rden[:])
                    pTT = pspool.tile([128, 64], F32, tag=tagT, name="pTT")
                    for h in range(2):
                        hsl = slice(h * 64, (h + 1) * 64)
                        pTr = pspool.tile([64, 64], BF16, tag=tagR, name="pTr")
                        nc.tensor.transpose(pTr[:], exn[hsl, :], c_idb[hsl, hsl])
                        aTh = sbp.tile([64, 64], BF16, tag="aT", name="aTh")
                        nc.vector.tensor_copy(aTh[:], pTr[:])
                        co = (h * 2 + bb) * 64
                        nc.tensor.matmul(pTT[hsl, :], vre[c2][:, co:co + 64],
                                         aTh[:], start=True, stop=True)
                    nc.vector.tensor_add(outT[c2][:, bsl], pTT[:, :],
                                         qT[c2][:, bsl])

                for sg in range(6):
                    j = sg // 2
                    # B: qkv projection for 3 chunks, group-major so all
                    # chunks consume freshly arrived xT columns
                    qks = [qkp.tile([128, ROWS], BF16, tag=f"qk{i}", name=f"qk{i}")
                           for i in range(3)]
                    wqas = []
                    for i in range(3):
                        wqa = wqp.tile([128, D], BF16, tag=f"wqa{i}",
                                       name=f"wqa{i}")
                        nc.gpsimd.dma_start(wqa[:], Wqc[sg * 3 + i])
                        wqas.append(wqa)
                    for grp in G3:
                        for i in range(3):
                            pts = []
                            for t, (goff, gsz) in enumerate(grp):
                                pts.append(psB.tile([128, gsz], F32,
                                                    tag=f"pb{t}", name=f"pb{t}"))
                            for t, (goff, gsz) in enumerate(grp):
                                for kk in range(KT):
                                    nc.tensor.matmul(
                                        pts[t][:], wqas[i][:, kk * 128:(kk + 1) * 128],
                                        xT[kk][:, goff:goff + gsz],
                                        start=(kk == 0), stop=(kk == KT - 1))
                            for t, (goff, gsz) in enumerate(grp):
                                if ei % 2 == 0:
                                    nc.vector.tensor_copy(
                                        qks[i][:, goff:goff + gsz], pts[t][:])
                                else:
                                    nc.scalar.copy(
                                        qks[i][:, goff:goff + gsz], pts[t][:])
                                ei += 1
                    # C: pooling conv, accumulate over hw
                    pas = [psC.tile([128, 128], F32, tag=f"pa{i}", name=f"pa{i}")
                           for i in range(3)]
                    if sg % 2 == 0:
                        wpa = wpp.tile([128, 3200], BF16, tag="wpa")
                        nc.gpsimd.dma_start(wpa[:], Wpb[j][:, 0:3200])
                        wpb2 = wpp.tile([128, 3072], BF16, tag="wpb2")
                        nc.gpsimd.dma_start(wpb2[:], Wpb[j][:, 3200:6272])
                        wpcache[j] = (wpa, wpb2)
                    else:
                        wpa, wpb2 = wpcache[j]
                    for hw in range(HW2):
                        if hw < 25:
                            wp = wpa[:, hw * 128:(hw + 1) * 128]
                        else:
                            wp = wpb2[:, (hw - 25) * 128:(hw - 24) * 128]
                        for i in range(3):
                            nc.tensor.matmul(
                                pas[i][:], wp,
                                qks[i][:, hw:ROWS:HW2],
                                start=(hw == 0), stop=(hw == HW2 - 1))
                    # D: bias + layernorm per chunk, inline
                    for i in range(3):
                        ch = sg * 3 + i
                        i6 = ch % 6
                        pt0 = ptp.tile([128, 128], F32, tag="pt0")
                        nc.vector.tensor_scalar_add(pt0[:], pas[i][:],
                                                    c_pb[:, j:j + 1])
                        p1 = psD.tile([128, 128], F32, tag="ptr")
                        nc.tensor.transpose(p1[:], pt0[:], c_id[:])
                        row = lnt.tile([128, 128], F32, tag="row")
                        if ch % 2 == 0:
                            nc.vector.tensor_copy(row[:], p1[:])
                        else:
                            nc.scalar.copy(row[:], p1[:])
                        rs = lnt.tile([128, 2], F32, tag="rs")
                        for h in range(2):
                            nc.vector.tensor_reduce(
                                rs[:, h:h + 1], row[:, h * 64:(h + 1) * 64],
                                axis=AX.X, op=OP.add)
                        mean = lnt.tile([128, 2], F32, tag="mean")
                        nc.vector.tensor_scalar_mul(mean[:], rs[:], 1.0 / 64)
                        cen = lnt.tile([128, 128], F32, tag="cen")
                        for h in range(2):
                            nc.vector.tensor_scalar_sub(
                                cen[:, h * 64:(h + 1) * 64],
                                row[:, h * 64:(h + 1) * 64], mean[:, h:h + 1])
                        sq = lnt.tile([128, 128], F32, tag="sq")
                        nc.vector.tensor_mul(sq[:], cen[:], cen[:])
                        vs = lnt.tile([128, 2], F32, tag="vs")
                        for h in range(2):
                            nc.vector.tensor_reduce(
                                vs[:, h:h + 1], sq[:, h * 64:(h + 1) * 64],
                                axis=AX.X, op=OP.add)
                        std = lnt.tile([128, 2], F32, tag="std")
                        nc.scalar.activation(std[:], vs[:], AF.Sqrt,
                                             bias=c_eps[:], scale=1.0 / 64)
                        rstd = lnt.tile([128, 2], F32, tag="rstd")
                        nc.vector.reciprocal(rstd[:], std[:])
                        nrm = lnt.tile([128, 128], F32, tag="nrm")
                        for h in range(2):
                            nc.vector.tensor_scalar_mul(
                                nrm[:, h * 64:(h + 1) * 64],
                                cen[:, h * 64:(h + 1) * 64], rstd[:, h:h + 1])
                        tmp = lnt.tile([128, 128], F32, tag="tmp")
                        nc.vector.tensor_mul(tmp[:], nrm[:],
                                             c_ag[:, j * 128:(j + 1) * 128])
                        if j == 2:
                            nc.vector.tensor_add(vrow[i6][:], tmp[:],
                                                 c_ab[:, j * 128:(j + 1) * 128])
                            for h in range(2):
                                for bb in range(2):
                                    co = (h * 2 + bb) * 64
                                    nc.sync.dma_start(
                                        vre[i6][:, co:co + 64],
                                        vrow[i6][bb * 64:(bb + 1) * 64,
                                                 h * 64:(h + 1) * 64])
                        else:
                            fin = lnt.tile([128, 128], F32, tag="fin")
                            nc.vector.tensor_add(fin[:], tmp[:],
                                                 c_ab[:, j * 128:(j + 1) * 128])
                            p2 = psD.tile([128, 128], F32, tag="ptr2")
                            nc.tensor.transpose(p2[:], fin[:], c_id[:])
                            dst = qT[i6] if j == 0 else kTt[i6]
                            if ch % 2 == 0:
                                nc.vector.tensor_copy(dst[:], p2[:])
                            else:
                                nc.scalar.copy(dst[:], p2[:])
                    if sg == 4:
                        for c2e in range(3):
                            for bbe in range(2):
                                emit_pass2(c2e, bbe, psD, "ptr2", "ptr", atp)
                    if sg == 3:
                        # attention pass 1: S = q.k, G = q.rpt (needs only q, k)
                        for c2 in range(6):
                            for bb in range(2):
                                bsl = slice(bb * 64, (bb + 1) * 64)
                                pSG = psD.tile([128, 192], F32, tag="ptr2", name="pSG")
                                for h in range(2):
                                    hsl = slice(h * 64, (h + 1) * 64)
                                    nc.tensor.matmul(pSG[hsl, 0:64],
                                                     qT[c2][hsl, bsl],
                                                     kTt[c2][hsl, bsl],
                                                     start=True, stop=True)
                                    nc.tensor.matmul(pSG[hsl, 64:191],
                                                     qT[c2][hsl, bsl],
                                                     c_rpt[hsl, :],
                                                     start=True, stop=True)
                                sSt = atp.tile([128, 64], F32, tag=f"sS{c2}_{bb}",
                                               name=f"sS{c2}_{bb}", bufs=1)
                                nc.vector.tensor_copy(sSt[:], pSG[:, 0:64])
                                sS.append(sSt)
                                gsb = atp.tile([128, 127], F32, tag="gsb")
                                nc.scalar.copy(gsb[:], pSG[:, 64:191])
                                gd = drp.tile([128, 127], F32, tag="gd")
                                nc.sync.dma_start(gd[:], gsb[:])
                                gds.append(gd)

            # ---------------- stage E: attention pass 2 + projection ----------------
            with (
                tc.tile_pool(name="psO", bufs=3, space="PSUM") as psO,
                tc.tile_pool(name="att", bufs=8) as att,
                tc.tile_pool(name="prj", bufs=1) as prj,
                tc.tile_pool(name="psY", bufs=1, space="PSUM") as psY,
            ):
                c_wpr = []
                for cc in range(KT):
                    w = prj.tile([128, D], BF16, tag=f"wpr{cc}", name=f"wpr{cc}")
                    nc.gpsimd.dma_start(w[:], Wpr[cc])
                    c_wpr.append(w)
                c_bpr = prj.tile([128, D], F32, tag="bpr")
                nc.gpsimd.dma_start(c_bpr[:], bprojR[:, :])
                ysb = prj.tile([128, D], F32, tag="ysb")
                for c2 in range(3, 6):
                    for bb in range(2):
                        emit_pass2(c2, bb, psO, "pTT", "pTr", att)

                # projection (accumulates per chunk as outT completes)
                for goff, gsz in [(0, 512), (512, 256)]:
                    pY = psY.tile([128, gsz], F32, tag="pY")
                    for cc in range(KT):
                        nc.tensor.matmul(pY[:], outT[cc][:],
                                         c_wpr[cc][:, goff:goff + gsz],
                                         start=(cc == 0), stop=(cc == KT - 1))
                    nc.vector.tensor_add(ysb[:, goff:goff + gsz], pY[:],
                                         c_bpr[:, goff:goff + gsz])
                    nc.sync.dma_start(yout[:, goff:goff + gsz],
                                      ysb[:, goff:goff + gsz])
            atp.release()

    nc.compile()
    return nc


def _host_prep(W_qkv, Wpq, bpq, Wpk, bpk, Wpv, bpv,
               g_q, be_q, g_k, be_k, g_v, be_v, rel_pos_t, W_proj, b_proj):
    bf = ml_dtypes.bfloat16
    Wqc = np.ascontiguousarray(
        np.asarray(W_qkv, np.float32).reshape(KT, 128, NCH, 128)
        .transpose(2, 1, 0, 3).reshape(NCH, 128, D)).astype(bf)
    Wpb = np.zeros((3, HW2, 128, 128), np.float32)
    for j, Wp in enumerate((Wpq, Wpk, Wpv)):
        Wp = np.asarray(Wp, np.float32)                      # (dout, cin, 7, 7)
        WpT = Wp.transpose(2, 3, 1, 0).reshape(HW2, C, C)    # (hw, ci, dout)
        Wpb[j, :, 0:64, 0:64] = WpT
        Wpb[j, :, 64:128, 64:128] = WpT
    Wpb = np.ascontiguousarray(
        Wpb.transpose(0, 2, 1, 3).reshape(3, 128, HW2 * 128)).astype(bf)
    rp = np.ascontiguousarray(np.asarray(rel_pos_t, np.float32)[::-1].T)  # (64,127)
    Rpt = np.concatenate([rp, rp], axis=0).astype(bf)        # (128,127)

    gq, gk, gv = (np.asarray(a, np.float32) for a in (g_q, g_k, g_v))
    bq, bk, bv = (np.asarray(a, np.float32) for a in (be_q, be_k, be_v))
    gk = gk * SCALE
    bk = bk * SCALE
    affG = np.broadcast_to(
        np.concatenate([np.tile(g, 2) for g in (gq, gk, gv)])[None, :],
        (128, 384)).copy()
    affB = np.broadcast_to(
        np.concatenate([np.tile(b, 2) for b in (bq, bk, bv)])[None, :],
        (128, 384)).copy()
    pbias = np.ascontiguousarray(np.stack(
        [np.tile(np.asarray(b, np.float32), 2) for b in (bpq, bpk, bpv)], axis=1))
    Wpr = np.ascontiguousarray(
        np.asarray(W_proj, np.float32).reshape(KT, 128, D)).astype(bf)
    bprojR = np.broadcast_to(
        np.asarray(b_proj, np.float32)[None, :], (128, D)).copy()
    idtf = np.eye(128, dtype=np.float32)
    return {"Wqc": Wqc, "Wpb": Wpb, "Rpt": Rpt, "affG": affG, "affB": affB,
            "pbias": pbias, "Wpr": Wpr, "bprojR": bprojR, "idtf": idtf,
            "idtb": idtf.astype(bf)}


def kernel(x, W_qkv, Wpq, bpq, Wpk, bpk, Wpv, bpv,
           g_q, be_q, g_k, be_k, g_v, be_v, rel_pos_t, W_proj, b_proj):
    global LAST_RESULT
    if "nc" not in _cache:
        _cache["nc"] = _build_nc()
    nc = _cache["nc"]

    shared = _host_prep(W_qkv, Wpq, bpq, Wpk, bpk, Wpv, bpv,
                        g_q, be_q, g_k, be_k, g_v, be_v,
                        rel_pos_t, W_proj, b_proj)
    bf = ml_dtypes.bfloat16
    xr = np.asarray(x, np.float32).reshape(B, T, HW2, D)
    in_maps = []
    for i in range(NCORES):
        xc = xr[i * BLOC:(i + 1) * BLOC].reshape(ROWS, D).astype(bf)
        xs = np.ascontiguousarray(xc.T).reshape(KT, 128, ROWS)
        m = {"xd": xs}
        m.update(shared)
        in_maps.append(m)

    res = run_bass_kernel_spmd(nc, in_maps, core_ids=list(range(NCORES)))
    LAST_RESULT = res
    y = np.stack([np.asarray(res.results[i]["y"], np.float32)
                  for i in range(NCORES)], axis=0)      # (8, 128, 768)
    return y.reshape(B, T, D).astype(np.float32)


# revision 6
# speedup vs baseline: 1.0855x; 1.0015x over previous
import numpy as np
import ml_dtypes
import bass_rust as _br

import concourse.bass as bass
import concourse.mybir as mybir
import concourse.tile as tile
from concourse import bacc
from concourse.bass_utils import run_bass_kernel_spmd
from concourse.tile_rust import add_dep_helper

F32 = mybir.dt.float32
BF16 = mybir.dt.bfloat16
AX = mybir.AxisListType
AF = mybir.ActivationFunctionType
OP = mybir.AluOpType

B, T, HWs, D, N, C = 16, 64, 7, 768, 12, 64
HW2 = HWs * HWs                 # 49
NCORES = 8
BLOC = B // NCORES              # 2
ROWS = BLOC * T * HW2           # 6272
NCOL = 3 * D                    # 2304
NCH = NCOL // 128               # 18 qkvT chunks
KT = D // 128                   # 6
EPS = 1e-5
SCALE = C ** -0.5
CGRPS = [(g * 512, 512) for g in range(12)] + [(6144, 128)]
G3 = [CGRPS[0:3], CGRPS[3:6], CGRPS[6:9], CGRPS[9:12], CGRPS[12:13]]
XCHUNKS = [(0, 512), (512, 1536), (2048, 2048), (4096, 2176)]

_cache = {}
LAST_RESULT = None


def _build_nc():
    nc = bacc.Bacc(None, target_bir_lowering=False, debug=False)
    xd = nc.declare_dram_parameter("xd", [KT, 128, ROWS], BF16, isOutput=False)
    Wqc = nc.declare_dram_parameter("Wqc", [NCH, 128, D], BF16, isOutput=False)
    Wpb = nc.declare_dram_parameter("Wpb", [3, 128, HW2 * 128], BF16, isOutput=False)
    Rpt = nc.declare_dram_parameter("Rpt", [128, 127], BF16, isOutput=False)
    affG = nc.declare_dram_parameter("affG", [128, 384], F32, isOutput=False)
    affB = nc.declare_dram_parameter("affB", [128, 384], F32, isOutput=False)
    pbias = nc.declare_dram_parameter("pbias", [128, 3], F32, isOutput=False)
    Wpr = nc.declare_dram_parameter("Wpr", [KT, 128, D], BF16, isOutput=False)
    bprojR = nc.declare_dram_parameter("bprojR", [128, D], F32, isOutput=False)
    idtf = nc.declare_dram_parameter("idtf", [128, 128], F32, isOutput=False)
    idtb = nc.declare_dram_parameter("idtb", [128, 128], BF16, isOutput=False)
    yout = nc.declare_dram_parameter("y", [128, D], F32, isOutput=True)

    wpcache = {}
    with tile.TileContext(nc) as tc:
        with (
            tc.tile_pool(name="cst", bufs=1) as cst,
            tc.tile_pool(name="qkv", bufs=1) as qkv,
            tc.tile_pool(name="drp", bufs=12, space="DRAM") as drp,
        ):
            c_rpt = cst.tile([128, 127], BF16, tag="rpt")
            nc.scalar.dma_start(c_rpt[:], Rpt[:, :])
            c_ag = cst.tile([128, 384], F32, tag="ag")
            nc.scalar.dma_start(c_ag[:], affG[:, :])
            c_ab = cst.tile([128, 384], F32, tag="ab")
            nc.scalar.dma_start(c_ab[:], affB[:, :])
            c_pb = cst.tile([128, 3], F32, tag="pb")
            nc.scalar.dma_start(c_pb[:], pbias[:, :])
            c_id = cst.tile([128, 128], F32, tag="idf")
            nc.scalar.dma_start(c_id[:], idtf[:, :])
            c_eps = cst.tile([128, 1], F32, tag="eps")
            nc.vector.memset(c_eps[:], EPS)
            c_idb = cst.tile([128, 128], BF16, tag="idb")
            nc.scalar.dma_start(c_idb[:], idtb[:, :])

            atp = tc.alloc_tile_pool(name="atp", bufs=6)
            qT = [qkv.tile([128, 128], BF16, tag=f"q{i}", name=f"q{i}") for i in range(6)]
            kTt = [qkv.tile([128, 128], BF16, tag=f"k{i}", name=f"k{i}") for i in range(6)]
            vrow = [qkv.tile([128, 128], BF16, tag=f"v{i}", name=f"v{i}") for i in range(6)]
            vre = [qkv.tile([64, 256], BF16, tag=f"vr{i}", name=f"vr{i}") for i in range(6)]
            outT = [qkv.tile([128, 128], BF16, tag=f"o{i}", name=f"o{i}") for i in range(6)]

            # ---------------- stages A/B/C/D fused ----------------
            with (
                tc.tile_pool(name="xtp", bufs=1) as xtp,
                tc.tile_pool(name="qkp", bufs=1) as qkp,
                tc.tile_pool(name="wqp", bufs=2) as wqp,
                tc.tile_pool(name="wpp", bufs=2) as wpp,
                tc.tile_pool(name="ptp", bufs=2) as ptp,
                tc.tile_pool(name="lnt", bufs=2) as lnt,
                tc.tile_pool(name="psB", bufs=1, space="PSUM") as psB,
                tc.tile_pool(name="psC", bufs=1, space="PSUM") as psC,
                tc.tile_pool(name="psD", bufs=1, space="PSUM") as psD,
            ):
                # A: load pre-transposed x directly (host supplies xT)
                xT = [xtp.tile([128, ROWS], BF16, tag=f"x{kk}", name=f"x{kk}")
                      for kk in range(KT)]
                for c0, c1 in [(0, 512), (512, 1568), (1568, 3136), (3136, 4704), (4704, ROWS)]:
                    for kk in range(KT):
                        nc.sync.dma_start(xT[kk][:, c0:c1], xd[kk][:, c0:c1])

                ei = 0
                sS = []
                gds = []

                def emit_pass2(c2, bb, pspool, tagT, tagR, sbp):
                    bsl = slice(bb * 64, (bb + 1) * 64)
                    idx = c2 * 2 + bb
                    rel = sbp.tile([128, 64], F32, tag="rel", name="rel")
                    src = gds[idx][:, :].copy()
                    src.ap = _br.VecI64Pair([[64 * 127, 2], [126, 64], [1, 64]])
                    src.offset = src.offset + 63
                    nc.sync.dma_start(rel[:, :], src)
                    s2 = sbp.tile([128, 64], F32, tag="s2", name="s2")
                    nc.vector.tensor_add(s2[:], sS[idx][:], rel[:])
                    ex = sbp.tile([128, 64], F32, tag="ex", name="ex")
                    den = sbp.tile([128, 1], F32, tag="den", name="den")
                    nc.scalar.activation(ex[:], s2[:], AF.Exp,
                                         bias=0.0, scale=1.0, accum_out=den[:])
                    rden = sbp.tile([128, 1], F32, tag="rden", name="rden")
                    nc.vector.reciprocal(rden[:], den[:])
                    exn = sbp.tile([128, 64], BF16, tag="exn", name="exn")
                    nc.vector.tensor_scalar_mul(exn[:], ex[:], rden[:])
                    pTT = pspool.tile([128, 64], F32, tag=tagT, name="pTT")
                    for h in range(2):
                        hsl = slice(h * 64, (h + 1) * 64)
                        pTr = pspool.tile([64, 64], BF16, tag=tagR, name="pTr")
                        nc.tensor.transpose(pTr[:], exn[hsl, :], c_idb[hsl, hsl])
                        aTh = sbp.tile([64, 64], BF16, tag="aT", name="aTh")
                        nc.vector.tensor_copy(aTh[:], pTr[:])
                        co = (h * 2 + bb) * 64
                        nc.tensor.matmul(pTT[hsl, :], vre[c2][:, co:co + 64],
                                         aTh[:], start=True, stop=True)
                    nc.vector.tensor_add(outT[c2][:, bsl], pTT[:, :],
                                         qT[c2][:, bsl])

                for sg in range(6):
                    j = sg // 2
                    # B: qkv projection for 3 chunks, group-major so all
                    # chunks consume freshly arrived xT columns
                    qks = [qkp.tile([128, ROWS], BF16, tag=f"qk{i}", name=f"qk{i}")
                           for i in range(3)]
                    wqas = []
                    for i in range(3):
                        wqa = wqp.tile([128, D], BF16, tag=f"wqa{i}",
                                       name=f"wqa{i}")
                        nc.gpsimd.dma_start(wqa[:], Wqc[sg * 3 + i])
                        wqas.append(wqa)
                    for grp in G3:
                        for i in range(3):
                            pts = []
                            for t, (goff, gsz) in enumerate(grp):
                                pts.append(psB.tile([128, gsz], F32,
                                                    tag=f"pb{t}", name=f"pb{t}"))
                            for t, (goff, gsz) in enumerate(grp):
                                for kk in range(KT):
                                    nc.tensor.matmul(
                                        pts[t][:], wqas[i][:, kk * 128:(kk + 1) * 128],
                                        xT[kk][:, goff:goff + gsz],
                                        start=(kk == 0), stop=(kk == KT - 1))
                            for t, (goff, gsz) in enumerate(grp):
                                if ei % 2 == 0:
                                    nc.vector.tensor_copy(
                                        qks[i][:, goff:goff + gsz], pts[t][:])
                                else:
                                    nc.scalar.copy(
                                        qks[i][:, goff:goff + gsz], pts[t][:])
                                ei += 1
                    # C: pooling conv, accumulate over hw
                    pas = [psC.tile([128, 128], F32, tag=f"pa{i}", name=f"pa{i}")
                           for i in range(3)]
                    if sg % 2 == 0:
                        wpa = wpp.tile([128, 3200], BF16, tag="wpa")
                        nc.gpsimd.dma_start(wpa[:], Wpb[j][:, 0:3200])
                        wpb2 = wpp.tile([128, 3072], BF16, tag="wpb2")
                        nc.gpsimd.dma_start(wpb2[:], Wpb[j][:, 3200:6272])
                        wpcache[j] = (wpa, wpb2)
                    else:
                        wpa, wpb2 = wpcache[j]
                    for hw in range(HW2):
                        if hw < 25:
                            wp = wpa[:, hw * 128:(hw + 1) * 128]
                        else:
                            wp = wpb2[:, (hw - 25) * 128:(hw - 24) * 128]
                        for i in range(3):
                            nc.tensor.matmul(
                                pas[i][:], wp,
                                qks[i][:, hw:ROWS:HW2],
                                start=(hw == 0), stop=(hw == HW2 - 1))
                    # D: bias + layernorm per chunk, inline
                    for i in range(3):
                        ch = sg * 3 + i
                        i6 = ch % 6
                        pt0 = ptp.tile([128, 128], F32, tag="pt0")
                        nc.vector.tensor_scalar_add(pt0[:], pas[i][:],
                                                    c_pb[:, j:j + 1])
                        p1 = psD.tile([128, 128], F32, tag="ptr")
                        nc.tensor.transpose(p1[:], pt0[:], c_id[:])
                        row = lnt.tile([128, 128], F32, tag="row")
                        if ch % 2 == 0:
                            nc.vector.tensor_copy(row[:], p1[:])
                        else:
                            nc.scalar.copy(row[:], p1[:])
                        rs = lnt.tile([128, 2], F32, tag="rs")
                        for h in range(2):
                            nc.vector.tensor_reduce(
                                rs[:, h:h + 1], row[:, h * 64:(h + 1) * 64],
                                axis=AX.X, op=OP.add)
                        mean = lnt.tile([128, 2], F32, tag="mean")
                        nc.vector.tensor_scalar_mul(mean[:], rs[:], 1.0 / 64)
                        cen = lnt.tile([128, 128], F32, tag="cen")
                        for h in range(2):
                            nc.vector.tensor_scalar_sub(
                                cen[:, h * 64:(h + 1) * 64],
                                row[:, h * 64:(h + 1) * 64], mean[:, h:h + 1])
                        sq = lnt.tile([128, 128], F32, tag="sq")
                        nc.vector.tensor_mul(sq[:], cen[:], cen[:])
                        vs = lnt.tile([128, 2], F32, tag="vs")
                        for h in range(2):
                            nc.vector.tensor_reduce(
                                vs[:, h:h + 1], sq[:, h * 64:(h + 1) * 64],
                                axis=AX.X, op=OP.add)
                        std = lnt.tile([128, 2], F32, tag="std")
                        nc.scalar.activation(std[:], vs[:], AF.Sqrt,
                                             bias=c_eps[:], scale=1.0 / 64)
                        rstd = lnt.tile([128, 2], F32, tag="rstd")
                        nc.vector.reciprocal(rstd[:], std[:])
                        nrm = lnt.tile([128, 128], F32, tag="nrm")
                        for h in range(2):
                            nc.vector.tensor_scalar_mul(
                                nrm[:, h * 64:(h + 1) * 64],
                                cen[:, h * 64:(h + 1) * 64], rstd[:, h:h + 1])
                        tmp = lnt.tile([128, 128], F32, tag="tmp")
                        nc.vector.tensor_mul(tmp[:], nrm[:],
                                             c_ag[:, j * 128:(j + 1) * 128])
                        if j == 2:
                            nc.vector.tensor_add(vrow[i6][:], tmp[:],
                                                 c_ab[:, j * 128:(j + 1) * 128])
                            for h in range(2):
                                for bb in range(2):
                                    co = (h * 2 + bb) * 64
                                    nc.sync.dma_start(
                                        vre[i6][:, co:co + 64],
                                        vrow[i6][bb * 64:(bb + 1) * 64,
                                                 h * 64:(h + 1) * 64])
                        else:
                            fin = lnt.tile([128, 128], F32, tag="fin")
                            nc.vector.tensor_add(fin[:], tmp[:],
                                                 c_ab[:, j * 128:(j + 1) * 128])
                            p2 = psD.tile([128, 128], F32, tag="ptr2")
                            nc.tensor.transpose(p2[:], fin[:], c_id[:])
                            dst = qT[i6] if j == 0 else kTt[i6]
                            if ch % 2 == 0:
                                nc.vector.tensor_copy(dst[:], p2[:])
                            else:
                                nc.scalar.copy(dst[:], p2[:])
                    if sg == 4:
                        for c2e in range(3):
                            for bbe in range(2):
                                emit_pass2(c2e, bbe, psD, "ptr2", "ptr", atp)
                    if sg == 3:
                        # attention pass 1: S = q.k, G = q.rpt (needs only q, k)
                        for c2 in range(6):
                            for bb in range(2):
                                bsl = slice(bb * 64, (bb + 1) * 64)
                                pSG = psD.tile([128, 192], F32, tag="ptr2", name="pSG")
                                for h in range(2):
                                    hsl = slice(h * 64, (h + 1) * 64)
                                    nc.tensor.matmul(pSG[hsl, 0:64],
                                                     qT[c2][hsl, bsl],
                                                     kTt[c2][hsl, bsl],
                                                     start=True, stop=True)
                                    nc.tensor.matmul(pSG[hsl, 64:191],
                                                     qT[c2][hsl, bsl],
                                                     c_rpt[hsl, :],
                                                     start=True, stop=True)
                                sSt = atp.tile([128, 64], F32, tag=f"sS{c2}_{bb}",
                                               name=f"sS{c2}_{bb}", bufs=1)
                                nc.vector.tensor_copy(sSt[:], pSG[:, 0:64])
                                sS.append(sSt)
                                gsb = atp.tile([128, 127], F32, tag="gsb")
                                nc.scalar.copy(gsb[:], pSG[:, 64:191])
                                gd = drp.tile([128, 127], F32, tag="gd")
                                nc.sync.dma_start(gd[:], gsb[:])
                                gds.append(gd)

            # ---------------- stage E: attention pass 2 + projection ----------------
            with (
                tc.tile_pool(name="psO", bufs=3, space="PSUM") as psO,
                tc.tile_pool(name="att", bufs=8) as att,
                tc.tile_pool(name="prj", bufs=1) as prj,
                tc.tile_pool(name="psY", bufs=2, space="PSUM") as psY,
            ):
                c_wpr = []
                for cc in range(KT):
                    w = prj.tile([128, D], BF16, tag=f"wpr{cc}", name=f"wpr{cc}")
                    nc.gpsimd.dma_start(w[:], Wpr[cc])
                    c_wpr.append(w)
                c_bpr = prj.tile([128, D], F32, tag="bpr")
                nc.gpsimd.dma_start(c_bpr[:], bprojR[:, :])
                ysb = prj.tile([128, D], F32, tag="ysb")
                for c2 in range(3, 6):
                    for bb in range(2):
                        emit_pass2(c2, bb, psO, "pTT", "pTr", att)

                # projection (accumulates per chunk as outT completes)
                for goff, gsz in [(0, 512), (512, 256)]:
                    pY = psY.tile([128, gsz], F32, tag="pY")
                    for cc in range(KT):
                        nc.tensor.matmul(pY[:], outT[cc][:],
                                         c_wpr[cc][:, goff:goff + gsz],
                                         start=(cc == 0), stop=(cc == KT - 1))
                    nc.vector.tensor_add(ysb[:, goff:goff + gsz], pY[:],
                                         c_bpr[:, goff:goff + gsz])
                    nc.sync.dma_start(yout[:, goff:goff + gsz],
                                      ysb[:, goff:goff + gsz])
            atp.release()

    nc.compile()
    return nc


def _host_prep(W_qkv, Wpq, bpq, Wpk, bpk, Wpv, bpv,
               g_q, be_q, g_k, be_k, g_v, be_v, rel_pos_t, W_proj, b_proj):
    bf = ml_dtypes.bfloat16
    Wqc = np.ascontiguousarray(
        np.asarray(W_qkv, np.float32).reshape(KT, 128, NCH, 128)
        .transpose(2, 1, 0, 3).reshape(NCH, 128, D)).astype(bf)
    Wpb = np.zeros((3, HW2, 128, 128), np.float32)
    for j, Wp in enumerate((Wpq, Wpk, Wpv)):
        Wp = np.asarray(Wp, np.float32)                      # (dout, cin, 7, 7)
        WpT = Wp.transpose(2, 3, 1, 0).reshape(HW2, C, C)    # (hw, ci, dout)
        Wpb[j, :, 0:64, 0:64] = WpT
        Wpb[j, :, 64:128, 64:128] = WpT
    Wpb = np.ascontiguousarray(
        Wpb.transpose(0, 2, 1, 3).reshape(3, 128, HW2 * 128)).astype(bf)
    rp = np.ascontiguousarray(np.asarray(rel_pos_t, np.float32)[::-1].T)  # (64,127)
    Rpt = np.concatenate([rp, rp], axis=0).astype(bf)        # (128,127)

    gq, gk, gv = (np.asarray(a, np.float32) for a in (g_q, g_k, g_v))
    bq, bk, bv = (np.asarray(a, np.float32) for a in (be_q, be_k, be_v))
    gk = gk * SCALE
    bk = bk * SCALE
    affG = np.broadcast_to(
        np.concatenate([np.tile(g, 2) for g in (gq, gk, gv)])[None, :],
        (128, 384)).copy()
    affB = np.broadcast_to(
        np.concatenate([np.tile(b, 2) for b in (bq, bk, bv)])[None, :],
        (128, 384)).copy()
    pbias = np.ascontiguousarray(np.stack(
        [np.tile(np.asarray(b, np.float32), 2) for b in (bpq, bpk, bpv)], axis=1))
    Wpr = np.ascontiguousarray(
        np.asarray(W_proj, np.float32).reshape(KT, 128, D)).astype(bf)
    bprojR = np.broadcast_to(
        np.asarray(b_proj, np.float32)[None, :], (128, D)).copy()
    idtf = np.eye(128, dtype=np.float32)
    return {"Wqc": Wqc, "Wpb": Wpb, "Rpt": Rpt, "affG": affG, "affB": affB,
            "pbias": pbias, "Wpr": Wpr, "bprojR": bprojR, "idtf": idtf,
            "idtb": idtf.astype(bf)}


def kernel(x, W_qkv, Wpq, bpq, Wpk, bpk, Wpv, bpv,
           g_q, be_q, g_k, be_k, g_v, be_v, rel_pos_t, W_proj, b_proj):
    global LAST_RESULT
    if "nc" not in _cache:
        _cache["nc"] = _build_nc()
    nc = _cache["nc"]

    shared = _host_prep(W_qkv, Wpq, bpq, Wpk, bpk, Wpv, bpv,
                        g_q, be_q, g_k, be_k, g_v, be_v,
                        rel_pos_t, W_proj, b_proj)
    bf = ml_dtypes.bfloat16
    xr = np.asarray(x, np.float32).reshape(B, T, HW2, D)
    in_maps = []
    for i in range(NCORES):
        xc = xr[i * BLOC:(i + 1) * BLOC].reshape(ROWS, D).astype(bf)
        xs = np.ascontiguousarray(xc.T).reshape(KT, 128, ROWS)
        m = {"xd": xs}
        m.update(shared)
        in_maps.append(m)

    res = run_bass_kernel_spmd(nc, in_maps, core_ids=list(range(NCORES)))
    LAST_RESULT = res
    y = np.stack([np.asarray(res.results[i]["y"], np.float32)
                  for i in range(NCORES)], axis=0)      # (8, 128, 768)
    return y.reshape(B, T, D).astype(np.float32)
